# revision 1
# baseline (speedup 1.0000x reference)
"""DeepseekMoE layer on 8 TRN2 NeuronCores — expert-parallel Bass/Tile kernel.

Strategy (self-contained, shapes hardcoded for this problem):
  H=2048, T=2048 tokens, E=16 experts, top-6, I=1408, shared IS=2816.

  Sharding (done on host inside kernel(), per the full-input contract):
    - Router (softmax + top-6) computed on host in fp32 (jax-on-CPU when
      available so near-tie selections match the jax reference bitwise)
      -> per-expert token lists (the "all-to-all dispatch" decision).
    - Core c owns experts 2c, 2c+1: receives w1/w2 transposed for those
      experts plus the gathered+transposed x columns of the tokens routed to
      them (capacity-padded to CAP), and the routing weights.
    - Shared expert is sharded over its intermediate dim: core c owns
      rows [352c, 352c+352) (padded to 384 = 3*128) of the shared MLP.
    - Each core returns per-expert outputs [CAP, H] (pre-scaled by routing
      weights) and a dense shared partial [T, H]; host scatter-adds.

  On-device per expert e (all matmuls fp32r = full PE rate, ~1.5e-4 rms):
    s1:  gate_up.T[o, t] = sum_h w1t[h, o] * xsel[h, t]
         silu fused into PSUM eviction; up-eviction is an in-place multiply
         -> act.T [i, t] in SBUF (fp32r)
    s2:  y[t, h] = sum_i act.T[i, t] * w2t[i, h], eviction fused with
         per-token routing-weight scale (ACT Copy, scale AP).
  Shared expert: identical structure over all T in 1024-token halves with
  its 24KB/partition down-projection weights kept resident.
  Overlap: each block's stage-2 second half is emitted after the next
  block's stage-1 (cross-block software pipeline over split s1/s2 PSUM
  pools), and deep output staging (6 bufs) keeps PSUM eviction off the
  store queue's critical path.
"""

import os
import sys

sys.path.insert(0, "/opt/trn_rl_repo")

import numpy as np

import concourse.bass as bass  # noqa: F401
import concourse.tile as tile
from concourse import bacc, mybir
from concourse.bass_utils import run_bass_kernel_spmd

H = 2048
T = 2048
E = 16
TOPK = 6
I2 = 2816  # 2*I
I = 1408
ISH = 2816  # shared intermediate (per gate/up half)
NCORES = 8
CAP0 = 896  # per-expert token capacity (avg load 768); grown if exceeded
SSL = 352  # shared-intermediate slice per core
SSLP = 384  # padded to 3*128

F32 = mybir.dt.float32
F32R = mybir.dt.float32r
AF = mybir.ActivationFunctionType

_compiled = {}
last_result = None  # BassKernelResults of the most recent run (for profiling)


def _nchunks(n):
    """Split n (multiple of 128) into fp32-matmul-friendly free-dim chunks:
    each <= 512 and >= 256 (fp32r runs 1 cyc/row only at N >= 256)."""
    out = []
    while n > 0:
        if n > 512:
            out.append(512)
            n -= 512
        elif n >= 256 or not out:
            out.append(n)
            n = 0
        else:  # n == 128: rebalance with previous 512 -> 384 + 256
            out[-1] -= 128
            out.append(256)
            n = 0
    return out


def _fine_chunks(ntok):
    """Chunk list with a small (256) first chunk — lets the first PSUM
    group start after a fraction of the x block has landed."""
    return [256] + _nchunks(ntok - 256)


def _emit_s1(nc, pools, *, w1t_ap, x_parts, act_tile, ntok, n_gate_ot,
             first_slab_hipri=False, chunks=None):
    """Stage 1: gate_up.T tiles, silu fused into eviction, in-place up-mul.

    w1t_ap:  DRAM [H, 2*n_gate_ot*128] (gate cols then up cols)
    x_parts: per token-chunk (sbuf_tile, col0) holding that chunk's x.T cols
    act_tile: SBUF [128, n_gate_ot, ntok] fp32r (written here)
    """
    w1p, psp = pools["w1"], pools["ps"]
    KT = 16  # h contraction tiles
    w1t_r = w1t_ap.rearrange("(k p) o -> p k o", p=128)
    spans = []
    t0 = 0
    for tcw in (chunks or _nchunks(ntok)):
        spans.append((t0, tcw))
        t0 += tcw
    assert len(x_parts) == len(spans)
    tc = pools["tc"]
    for ot in range(2 * n_gate_ot):
        w1slab = w1p.tile([128, KT, 128], F32R, tag="w1slab")
        if ot == 0 and first_slab_hipri:
            with tc.high_priority():
                nc.sync.dma_start(out=w1slab[:],
                                  in_=w1t_r[:, :, ot * 128:(ot + 1) * 128])
        else:
            nc.sync.dma_start(out=w1slab[:],
                              in_=w1t_r[:, :, ot * 128:(ot + 1) * 128])
        # k outer / chunk inner: consecutive matmuls reuse the stationary
        # operand w1slab[:, k, :], amortizing its LDWEIGHTS
        pss = [psp.tile([128, 512], F32, tag="ps", name=f"ps1_{ot}_{ci}")
               for ci in range(len(spans))]
        for k in range(KT):
            for ci, (t0, tcw) in enumerate(spans):
                xpt, xc0 = x_parts[ci]
                nc.tensor.matmul(
                    pss[ci][:, :tcw],
                    w1slab[:, k, :],
                    xpt[:, k, xc0:xc0 + tcw],
                    start=(k == 0),
                    stop=(k == KT - 1),
                )
        for ci, (t0, tcw) in enumerate(spans):
            if ot < n_gate_ot:
                nc.scalar.activation(
                    out=act_tile[:, ot, t0:t0 + tcw],
                    in_=pss[ci][:, :tcw],
                    func=AF.Silu,
                )
            else:
                sl = act_tile[:, ot - n_gate_ot, t0:t0 + tcw]
                nc.vector.tensor_mul(sl, pss[ci][:, :tcw], sl)


def _emit_s2(nc, pools, *, act_tile, w2t_ap, out_ap, out_row0, ntok,
             n_gate_ot, cw_tile, cw_col0=0, resident_w2=None, part=0):
    """Stage 2: down proj, per-token scale fused into eviction.

    w2t_ap:  DRAM [n_gate_ot*128, H]
    out_ap:  DRAM output, rows [out_row0, out_row0+ntok), all H cols
    cw_tile: SBUF [128, >=cw_col0+ntok/128] per-token scale, or None
    resident_w2: optional pre-loaded SBUF [128, n_gate_ot, H] weight tile
    """
    w2p, psp, outp = pools["w2"], pools["ps2"], pools["out"]
    s2_k = n_gate_ot
    w2t_r = w2t_ap.rearrange("(k p) h -> p k h", p=128)
    stash = pools.setdefault("w2stash", {})

    def get_slab(hc):
        key = (id(w2t_ap), out_row0, hc)
        if key in stash:
            return stash.pop(key)
        w2slab = w2p.tile([128, s2_k, 512], F32R, tag="w2slab",
                          name=f"w2slab_{out_row0}_{hc}")
        nc.sync.dma_start(out=w2slab[:],
                          in_=w2t_r[:, :, hc * 512:(hc + 1) * 512])
        return w2slab

    def prefetch_slab(hc):
        stash[(id(w2t_ap), out_row0, hc)] = get_slab(hc)
    ntt = ntok // 128
    tt_list = {0: range(ntt), 1: range(ntt // 2), 2: range(ntt // 2, ntt)}[part]
    hc_list = {0: range(4), 1: range(2), 2: range(2, 4)}[part]
    if resident_w2 is not None:
        # tt outer / hc inner: the stationary act[:, k, tt] is reused across
        # all four hc matmuls, amortizing its LDWEIGHTS 4x
        for tt in tt_list:
            pss = [psp.tile([128, 512], F32, tag="ps2", name=f"ps2r_{tt}_{hc}")
                   for hc in range(4)]
            for k in range(s2_k):
                for hc in range(4):
                    nc.tensor.matmul(
                        pss[hc][:],
                        act_tile[:, k, tt * 128:(tt + 1) * 128],
                        resident_w2[:, k, hc * 512:(hc + 1) * 512],
                        start=(k == 0),
                        stop=(k == s2_k - 1),
                    )
            for hc in range(4):
                ysb = outp.tile([128, 512], F32, tag="ysb",
                                name=f"ysbr_{tt}_{hc}")
                # alternate evict engine: ACT and DVE each drain two PSUM
                # groups per tt, halving the slot-recycle critical path
                if hc % 2 == 0:
                    nc.scalar.activation(out=ysb[:], in_=pss[hc][:], func=AF.Copy)
                else:
                    nc.vector.tensor_copy(ysb[:], pss[hc][:])
                nc.sync.dma_start(
                    out=out_ap[out_row0 + tt * 128: out_row0 + (tt + 1) * 128,
                               hc * 512:(hc + 1) * 512],
                    in_=ysb[:],
                )
        return
    for hc in hc_list:
        w2slab = get_slab(hc)
        for tt in range(ntok // 128):
            ps = psp.tile([128, 512], F32, tag="ps2", name=f"ps2_{hc}_{tt}")
            for k in range(s2_k):
                nc.tensor.matmul(
                    ps[:],
                    act_tile[:, k, tt * 128:(tt + 1) * 128],
                    w2slab[:, k, :],
                    start=(k == 0),
                    stop=(k == s2_k - 1),
                )
            ysb = outp.tile([128, 512], F32, tag="ysb", name=f"ysb_{hc}_{tt}")
            if cw_tile is not None:
                nc.scalar.activation(
                    out=ysb[:], in_=ps[:], func=AF.Copy,
                    scale=cw_tile[:, cw_col0 + tt:cw_col0 + tt + 1])
            else:
                nc.scalar.activation(out=ysb[:], in_=ps[:], func=AF.Copy)
            nc.sync.dma_start(
                out=out_ap[out_row0 + tt * 128: out_row0 + (tt + 1) * 128,
                           hc * 512:(hc + 1) * 512],
                in_=ysb[:],
            )



def _build(cap):
    nc = bacc.Bacc("TRN2", target_bir_lowering=False, debug=False)

    aps = {}
    for j in range(2):
        aps[f"xs{j}"] = nc.dram_tensor(f"xs{j}", [H, cap], F32R, kind="ExternalInput").ap()
        aps[f"w1t{j}"] = nc.dram_tensor(f"w1t{j}", [H, I2], F32R, kind="ExternalInput").ap()
        aps[f"w2t{j}"] = nc.dram_tensor(f"w2t{j}", [I, H], F32R, kind="ExternalInput").ap()
        aps[f"cw{j}"] = nc.dram_tensor(f"cw{j}", [cap], F32, kind="ExternalInput").ap()
        aps[f"y{j}"] = nc.dram_tensor(f"y{j}", [cap, H], F32, kind="ExternalOutput").ap()
    aps["xt"] = nc.dram_tensor("xt", [H, T], F32R, kind="ExternalInput").ap()
    aps["sw1t"] = nc.dram_tensor("sw1t", [H, 2 * SSLP], F32R, kind="ExternalInput").ap()
    aps["sw2t"] = nc.dram_tensor("sw2t", [SSLP, H], F32R, kind="ExternalInput").ap()
    aps["ys"] = nc.dram_tensor("ys", [T, H], F32, kind="ExternalOutput").ap()

    # token blocks per expert (<=1024 each, multiples of 128)
    eblocks = []
    r0 = 0
    while r0 < cap:
        w = min(1024, cap - r0)
        eblocks.append((r0, w))
        r0 += w

    import contextlib
    with tile.TileContext(nc) as tc, contextlib.ExitStack() as ctx:
        pools = {
            "x": ctx.enter_context(tc.tile_pool(name="x", bufs=1)),
            # cap > 896 grows the x slot to 64KB/partition; shed one w1
            # prefetch buffer to stay inside SBUF on that fallback path
            "w1": ctx.enter_context(tc.tile_pool(name="w1",
                                                 bufs=4 if cap <= 896 else 2)),
            "w2": ctx.enter_context(tc.tile_pool(name="w2", bufs=2)),
            "act": ctx.enter_context(tc.tile_pool(name="act", bufs=1)),
            "out": ctx.enter_context(tc.tile_pool(name="out", bufs=6)),
            # separate s1/s2 PSUM pools: the cross-block s2 deferral must
            # never be starved of PSUM slots by the next block's stalled s1
            "ps": ctx.enter_context(tc.tile_pool(name="ps", bufs=4, space="PSUM")),
            "ps2": ctx.enter_context(tc.tile_pool(name="ps2", bufs=4, space="PSUM")),
            "misc": ctx.enter_context(tc.tile_pool(name="misc", bufs=2)),
        }

        pools["tc"] = tc
        cw_tiles = {}

        def get_cw(j):  # lazy: cw loads shouldn't precede compute-critical DMAs
            if j not in cw_tiles:
                cw_r = aps[f"cw{j}"].rearrange("(n p) -> p n", p=128)
                cw_tiles[j] = pools["misc"].tile([128, cap // 128], F32,
                                                 tag=f"cw{j}", name=f"cw{j}_t")
                nc.sync.dma_start(out=cw_tiles[j][:], in_=cw_r[:])
            return cw_tiles[j]

        # Block order [e0, sh0, sh1, e1]:
        # - the big xt (shared) transfers land on the clean early boundaries
        #   where the previous block's stage-1 finishes on time
        # - the kernel ends on an expert block, whose store rate stays below
        #   its PE rate, shrinking the end-of-kernel store drain
        def expert_blocks(j):
            xs_r = aps[f"xs{j}"].rearrange("(k p) t -> p k t", p=128)
            return [dict(
                x_src=xs_r[:, :, row0:row0 + ntok], ntok=ntok, n_gate_ot=11,
                w1t_ap=aps[f"w1t{j}"], w2t_ap=aps[f"w2t{j}"],
                out_ap=aps[f"y{j}"], out_row0=row0,
                cw_j=j, cw_col0=row0 // 128, slot="xsel",
            ) for (row0, ntok) in eblocks]

        xt_r = aps["xt"].rearrange("(k p) t -> p k t", p=128)
        shared_blocks = [dict(
            x_src=xt_r[:, :, half * 1024:(half + 1) * 1024], ntok=1024,
            n_gate_ot=3, w1t_ap=aps["sw1t"], w2t_ap=aps["sw2t"],
            out_ap=aps["ys"], out_row0=half * 1024,
            cw_j=None, cw_col0=0, slot="xsel",
        ) for half in range(2)]

        blocks = expert_blocks(0) + expert_blocks(1) + shared_blocks

        def load_x(b, chunks, hipri_first_only=False):
            # chunked at the s1 span boundaries: each s1 PSUM group starts
            # as soon as its own columns have landed. Shared blocks put
            # chunk 0 in a dependency-free aux slot so the next block's
            # stage-1 can start the moment the previous one ends.
            parts = []
            xt_tile = pools["x"].tile([128, 16, b["ntok"]], F32R, tag=b["slot"],
                                      name=f"x_{b['slot']}_{b['out_row0']}")
            t0 = 0
            for ci, tcw in enumerate(chunks):
                if ci == 0 or not hipri_first_only:
                    with tc.high_priority():
                        nc.sync.dma_start(out=xt_tile[:, :, t0:t0 + tcw],
                                          in_=b["x_src"][:, :, t0:t0 + tcw])
                else:
                    nc.sync.dma_start(out=xt_tile[:, :, t0:t0 + tcw],
                                      in_=b["x_src"][:, :, t0:t0 + tcw])
                parts.append((xt_tile, t0))
                t0 += tcw
            return parts

        # Emit s1(n), then block n+1's x-load, then s2(n): the next x-load
        # lands ahead of s2(n)'s weight slabs in the scheduler's priority
        # order, so its (large) transfer overlaps s2(n) compute instead of
        # queueing behind it in the DGE FIFO.
        preloaded_aux = {}
        shared_w2_res = [None]

        def load_aux(b):
            aux = pools["x"].tile([128, 16, 512], F32R, tag="xaux",
                                  name=f"xaux_{b['out_row0']}")
            nc.sync.dma_start(out=aux[:], in_=b["x_src"][:, :, 0:512])
            return aux

        def s1_chunks(n):
            return (_fine_chunks(blocks[n]["ntok"]) if n == 0
                    else _nchunks(blocks[n]["ntok"]))

        def emit_s2_part(b, act_tile, part):
            res_w2 = None
            if b["cw_j"] is None:  # shared expert: 24KB w2 slice kept resident
                if shared_w2_res[0] is None:
                    rt = pools["w2"].tile([128, 3, H], F32R, tag="w2slab",
                                          name="sw2_resident")
                    nc.sync.dma_start(
                        out=rt[:],
                        in_=b["w2t_ap"].rearrange("(k p) h -> p k h", p=128))
                    shared_w2_res[0] = rt
                res_w2 = shared_w2_res[0]
            _emit_s2(nc, pools, act_tile=act_tile, w2t_ap=b["w2t_ap"],
                     out_ap=b["out_ap"], out_row0=b["out_row0"],
                     ntok=b["ntok"], n_gate_ot=b["n_gate_ot"],
                     cw_tile=None if b["cw_j"] is None else get_cw(b["cw_j"]),
                     cw_col0=b["cw_col0"], resident_w2=res_w2, part=part)

        x_tiles = [load_x(blocks[0], s1_chunks(0), hipri_first_only=True)]
        deferred = None
        for n, b in enumerate(blocks):
            act_tile = pools["act"].tile([128, b["n_gate_ot"], b["ntok"]],
                                         F32R, tag="act")
            _emit_s1(nc, pools, w1t_ap=b["w1t_ap"], x_parts=x_tiles[n],
                     act_tile=act_tile, ntok=b["ntok"],
                     n_gate_ot=b["n_gate_ot"], first_slab_hipri=True,
                     chunks=s1_chunks(n))
            if n + 1 < len(blocks):
                x_tiles.append(load_x(blocks[n + 1], s1_chunks(n + 1)))
            # cross-block software pipeline: the previous block's deferred
            # s2 half sits after this block's s1 in priority order, so the
            # scheduler can fill this block's x/slab wait with it
            if deferred is not None:
                emit_s2_part(*deferred, part=2)
                deferred = None
            emit_s2_part(b, act_tile, part=1)
            deferred = (b, act_tile)
        if deferred is not None:
            emit_s2_part(*deferred, part=2)

    nc.compile()
    return nc


def _route(xf, gate_w):
    """Host router: fp32 softmax + top-6.

    Uses jax on CPU when available so selection/weights match the jax
    reference bit-for-bit (matters only for near-exact prob ties).
    """
    try:
        import jax
        import jax.numpy as jnp

        cpu = jax.devices("cpu")[0]
        with jax.default_device(cpu):
            logits = jnp.asarray(xf) @ jnp.asarray(gate_w).T
            probs = jax.nn.softmax(logits.astype(jnp.float32), axis=-1)
            _, sel = jax.lax.top_k(probs, TOPK)
        return np.asarray(probs), np.asarray(sel)
    except Exception:
        logits = xf @ gate_w.T  # [T, E] fp32
        m = logits.max(axis=-1, keepdims=True)
        e = np.exp(logits - m, dtype=np.float32)
        probs = e / e.sum(axis=-1, keepdims=True)
        sel = np.argsort(-probs, axis=-1, kind="stable")[:, :TOPK]
        return probs, sel


def kernel(x, gate_w, w1, w2, shared_w1, shared_w2):
    x = np.asarray(x, np.float32)
    gate_w = np.asarray(gate_w, np.float32)
    w1 = np.asarray(w1, np.float32)
    w2 = np.asarray(w2, np.float32)
    shared_w1 = np.asarray(shared_w1, np.float32)
    shared_w2 = np.asarray(shared_w2, np.float32)

    B, S, Hd = x.shape
    xf = np.ascontiguousarray(x.reshape(-1, Hd))  # [T, H]

    probs, sel = _route(xf, gate_w)
    onehot = np.zeros((T, E), bool)
    onehot[np.arange(T)[:, None], sel] = True
    idx_e = [np.nonzero(onehot[:, e])[0] for e in range(E)]
    counts = np.array([len(ix) for ix in idx_e])

    cap = CAP0
    while counts.max() > cap:
        cap += 128
    if cap not in _compiled:
        _compiled[cap] = _build(cap)
    nc = _compiled[cap]

    xt = np.ascontiguousarray(xf.T)  # [H, T]

    in_maps = []
    for c in range(NCORES):
        m = {"xt": xt}
        for j in range(2):
            e = 2 * c + j
            ix = idx_e[e]
            xs = np.zeros((cap, H), np.float32)
            xs[: len(ix)] = xf[ix]
            m[f"xs{j}"] = np.ascontiguousarray(xs.T)
            m[f"w1t{j}"] = np.ascontiguousarray(w1[e].T)
            m[f"w2t{j}"] = np.ascontiguousarray(w2[e].T)
            cw = np.zeros(cap, np.float32)
            cw[: len(ix)] = probs[ix, e]
            m[f"cw{j}"] = cw
        sw1t = np.zeros((H, 2 * SSLP), np.float32)
        sw1t[:, :SSL] = shared_w1[SSL * c: SSL * (c + 1)].T
        sw1t[:, SSLP: SSLP + SSL] = shared_w1[ISH + SSL * c: ISH + SSL * (c + 1)].T
        m["sw1t"] = sw1t
        sw2t = np.zeros((SSLP, H), np.float32)
        sw2t[:SSL] = shared_w2[:, SSL * c: SSL * (c + 1)].T
        m["sw2t"] = sw2t
        in_maps.append(m)

    try:
        res = run_bass_kernel_spmd(nc, in_maps, list(range(NCORES)))
    except ModuleNotFoundError:
        # BASS_TRACE=1 requires the axon NTFF hook (antenv.axon_hooks),
        # absent in some containers — retry with tracing disabled.
        os.environ["BASS_NEVER_TRACE"] = "1"
        res = run_bass_kernel_spmd(nc, in_maps, list(range(NCORES)))
    global last_result
    last_result = res

    out = np.zeros((T, H), np.float32)
    for c in range(NCORES):
        out += res.results[c]["ys"]
        for j in range(2):
            e = 2 * c + j
            ix = idx_e[e]
            out[ix] += res.results[c][f"y{j}"][: len(ix)]

    return out.reshape(B, S, Hd)



# revision 6
# speedup vs baseline: 1.3544x; 1.3544x over previous
"""DeepseekMoE layer on 8 TRN2 NeuronCores — expert-parallel Bass/Tile kernel.

Strategy (self-contained, shapes hardcoded for this problem):
  H=2048, T=2048 tokens, E=16 experts, top-6, I=1408, shared IS=2816.

  Sharding (done on host inside kernel(), per the full-input contract):
    - Router (softmax + top-6) computed on host in fp32 (jax-on-CPU when
      available so near-tie selections match the jax reference bitwise)
      -> per-expert token lists (the "all-to-all dispatch" decision).
    - Core c owns experts 2c, 2c+1 (capacity-padded to CAP tokens each);
      shared expert sharded over its intermediate dim (352 rows per core,
      padded to 384 = 3*128).
    - Each core returns per-expert outputs [CAP, H] bf16 (pre-scaled by
      routing weights) and a dense shared partial [T, H] bf16; the host
      scatter-adds in fp32.

  Arithmetic: all matmuls run as fp8(e4m3) DoubleRow pairs at 0.5 cyc/row,
  using a hi+lo residual decomposition of every operand:
      a*s ~= a_hi + a_lo   (a_hi = fp8(a*s), a_lo = fp8(a*s - a_hi))
      a*b*s_a*s_b ~= a_hi*b_hi + a_lo*b_hi + a_hi*b_lo     (lo*lo dropped)
  Three DoubleRow instructions per k-tile pair = 0.75 cyc/row/k-tile, a
  1.33x speedup over bf16/fp32r with ~bf16 effective precision (measured
  end-to-end rel err 2.6e-3 vs the 2e-2 gate).

  Scales: x*4, w1*64, w2*128. Stage-1 PSUM: gate = g*256 (Silu evicted with
  scale 2^-8); up = u*256, fused DVE (ps_u * 1/16) * silu_g = act*16, which
  is split hi/lo to fp8 for stage 2. Stage-2 PSUM = y*2048; eviction scale
  folds 2^-11 into the per-token routing weight (or a constant for the
  shared expert). Odd k-tile counts (11 expert, 3 shared) are handled by
  host-side slab rows: hi-slab gets a duplicated last k-tile, lo-slab gets
  zeros, plus a one-time memset of the act tile's pad k-tile.
"""

import os
import sys

sys.path.insert(0, "/opt/trn_rl_repo")

import numpy as np
import ml_dtypes

import concourse.bass as bass  # noqa: F401
import concourse.tile as tile
from concourse import bacc, mybir
from concourse.bass_utils import run_bass_kernel_spmd

H = 2048
T = 2048
E = 16
TOPK = 6
I2 = 2816  # 2*I
I = 1408
ISH = 2816  # shared intermediate (per gate/up half)
NCORES = 8
CAP0 = 832  # per-expert token capacity; grown in 64s if exceeded
SSL = 352  # shared-intermediate slice per core
SSLP = 384  # padded to 3*128

SX, SW1, SW2, SACT = 4.0, 64.0, 128.0, 16.0
S1_EVICT = 1.0 / (SW1 * SX)  # 2^-8
S1_UP = SACT / (SW1 * SX)  # 1/16
S2_EVICT = 1.0 / (SACT * SW2)  # 2^-11

F8NP = ml_dtypes.float8_e4m3
BF16NP = ml_dtypes.bfloat16
F32 = mybir.dt.float32
F8 = mybir.dt.float8e4
BF16 = mybir.dt.bfloat16
AF = mybir.ActivationFunctionType
ALU = mybir.AluOpType
DR = mybir.MatmulPerfMode.DoubleRow

_compiled = {}
last_result = None  # BassKernelResults of the most recent run (for profiling)


def _nchunks(n, first=None):
    """Split n into <=512 free-dim chunks; optional smaller first chunk so the
    first PSUM group starts after a fraction of the x block has landed."""
    out = [first] if first else []
    n -= first or 0
    while n > 0:
        w = min(512, n)
        out.append(w)
        n -= w
    return out


def _emit_mm3(nc, ps, wh_slab, wl_slab, x_tile, xc0, w, n_kt, first, last):
    """Accumulate sum_k w~[k].T @ x~[k] into ps[:, :w] via 3-product fp8
    DoubleRow instructions. wh/wl slabs: [128, n_kt(+pad), 128]; x_tile:
    [128, n_kt, 2, tokens] (hi/lo interleaved), token cols [xc0, xc0+w).
    n_kt may be odd: slabs carry a dup/zero pad k-tile (see module doc)."""
    xs = x_tile[:, :, :, xc0:xc0 + w]
    npair = n_kt // 2
    n3 = npair * 3 + (2 if n_kt % 2 else 0)
    i = 0
    for kp in range(npair):
        k = 2 * kp
        for lhsT, rhs in (
            (wh_slab[:, k:k + 2, :], xs[:, k:k + 2, 0, :]),
            (wh_slab[:, k:k + 2, :], xs[:, k:k + 2, 1, :]),
            (wl_slab[:, k:k + 2, :], xs[:, k:k + 2, 0, :]),
        ):
            nc.tensor.matmul(ps[:, :w], lhsT, rhs,
                             start=(first and i == 0),
                             stop=(last and i == n3 - 1), perf_mode=DR)
            i += 1
    if n_kt % 2:
        k = n_kt - 1
        # (w_hi[k], w_hi[k]-dup) x (x_hi[k], x_lo[k])
        nc.tensor.matmul(ps[:, :w], wh_slab[:, k:k + 2, :], xs[:, k, :, :],
                         start=(first and i == 0), stop=False, perf_mode=DR)
        # (w_lo[k], zeros) x (x_hi[k], x_hi[k+1]-garbage*0)
        nc.tensor.matmul(ps[:, :w], wl_slab[:, k:k + 2, :],
                         xs[:, k:k + 2, 0, :],
                         start=False, stop=last, perf_mode=DR)


def _emit_s1(nc, pools, *, b, x_tile, act_tile, chunks):
    """Stage 1: per gate-o-tile, compute ps_g/ps_u via 3-product DoubleRow,
    then evict: ACT Silu -> ag; DVE (ps_u/16)*ag -> stage; ACT cast -> act_hi;
    DVE stage-hi -> act_lo. act_tile: [128, act_kt, 2, ntok] fp8."""
    w1p, psp, stp = pools["w1"], pools["ps"], pools["stage"]
    tc = pools["tc"]
    n_go = b["n_go"]
    w1h_r = b["w1h"].rearrange("(k p) o -> p k o", p=128)
    w1l_r = b["w1l"].rearrange("(k p) o -> p k o", p=128)
    spans = []
    t0 = 0
    for w in chunks:
        spans.append((t0, w))
        t0 += w
    for o in range(n_go):
        slabs = []
        for ot in (o, n_go + o):
            for src in (w1h_r, w1l_r):
                sl = w1p.tile([128, 16, 128], F8, tag="w1slab",
                              name=f"w1_{b['tag']}_{ot}_{id(src) % 97}")
                if o == 0 and b.get("hipri_slab"):
                    with tc.high_priority():
                        nc.sync.dma_start(
                            out=sl[:], in_=src[:, :, ot * 128:(ot + 1) * 128])
                else:
                    nc.sync.dma_start(
                        out=sl[:], in_=src[:, :, ot * 128:(ot + 1) * 128])
                slabs.append(sl)
        gwh, gwl, uwh, uwl = slabs
        for ci, (t0, w) in enumerate(spans):
            ps_g = psp.tile([128, 512], F32, tag="ps", name=f"psg_{o}_{ci}")
            ps_u = psp.tile([128, 512], F32, tag="ps", name=f"psu_{o}_{ci}")
            _emit_mm3(nc, ps_g, gwh, gwl, x_tile, t0, w, 16, True, True)
            _emit_mm3(nc, ps_u, uwh, uwl, x_tile, t0, w, 16, True, True)
            ag = stp.tile([128, 512], F32, tag="stage", name=f"ag_{o}_{ci}")
            st = stp.tile([128, 512], F32, tag="stage", name=f"st_{o}_{ci}")
            nc.scalar.activation(out=ag[:, :w], in_=ps_g[:, :w], func=AF.Silu,
                                 scale=S1_EVICT)
            nc.vector.scalar_tensor_tensor(
                out=st[:, :w], in0=ps_u[:, :w], scalar=S1_UP, in1=ag[:, :w],
                op0=ALU.mult, op1=ALU.mult)
            hi = act_tile[:, o, 0, t0:t0 + w]
            nc.scalar.activation(out=hi, in_=st[:, :w], func=AF.Copy)
            nc.vector.tensor_sub(act_tile[:, o, 1, t0:t0 + w], st[:, :w], hi)


def _emit_s2(nc, pools, *, b, act_tile, part):
    """Stage 2: out[t, hc] = sum_k act~[k].T @ w2~[k], 3-product DoubleRow
    with act (hi/lo) stationary and w2 slabs moving. Evict with per-token
    (expert) or constant (shared) scale to bf16, DMA out."""
    w2p, psp, outp = pools["w2"], pools["ps2"], pools["out"]
    n_kt = b["act_kt"] - 1 if b["odd_kt"] else b["act_kt"]
    ntok = b["ntok"]
    w2h_r = b["w2h"].rearrange("(k p) h -> p k h", p=128)
    w2l_r = b["w2l"].rearrange("(k p) h -> p k h", p=128)
    hc_list = {1: range(2), 2: range(2, 4)}[part]
    ntt = (ntok + 127) // 128
    for hc in hc_list:
        if b.get("w2_res") is not None:
            if b["w2_res"][hc] is None:
                sh = w2p.tile([128, b["act_kt"], 512], F8, tag="w2res",
                              bufs=8, name=f"w2h_res_{hc}")
                sl = w2p.tile([128, b["act_kt"], 512], F8, tag="w2res",
                              bufs=8, name=f"w2l_res_{hc}")
                nc.sync.dma_start(out=sh[:],
                                  in_=w2h_r[:, :, hc * 512:(hc + 1) * 512])
                nc.sync.dma_start(out=sl[:],
                                  in_=w2l_r[:, :, hc * 512:(hc + 1) * 512])
                b["w2_res"][hc] = (sh, sl)
            w2h_slab, w2l_slab = b["w2_res"][hc]
        else:
            w2h_slab = w2p.tile([128, b["act_kt"], 512], F8, tag="w2slab",
                                name=f"w2h_{b['tag']}_{hc}")
            w2l_slab = w2p.tile([128, b["act_kt"], 512], F8, tag="w2slab",
                                name=f"w2l_{b['tag']}_{hc}")
            nc.sync.dma_start(out=w2h_slab[:],
                              in_=w2h_r[:, :, hc * 512:(hc + 1) * 512])
            nc.sync.dma_start(out=w2l_slab[:],
                              in_=w2l_r[:, :, hc * 512:(hc + 1) * 512])
        for tt in range(ntt):
            r0 = tt * 128
            w = min(128, ntok - r0)
            ps = psp.tile([128, 512], F32, tag="ps2", name=f"ps2_{hc}_{tt}")
            npair = n_kt // 2
            n3 = npair * 3 + (2 if n_kt % 2 else 0)
            i = 0
            for kp in range(npair):
                k = 2 * kp
                for lhsT, rhs in (
                    (act_tile[:, k:k + 2, 0, r0:r0 + w], w2h_slab[:, k:k + 2, :]),
                    (act_tile[:, k:k + 2, 1, r0:r0 + w], w2h_slab[:, k:k + 2, :]),
                    (act_tile[:, k:k + 2, 0, r0:r0 + w], w2l_slab[:, k:k + 2, :]),
                ):
                    nc.tensor.matmul(ps[:w, :], lhsT, rhs, start=(i == 0),
                                     stop=(i == n3 - 1), perf_mode=DR)
                    i += 1
            if n_kt % 2:
                k = n_kt - 1
                nc.tensor.matmul(ps[:w, :], act_tile[:, k, :, r0:r0 + w],
                                 w2h_slab[:, k:k + 2, :],
                                 start=False, stop=False, perf_mode=DR)
                nc.tensor.matmul(ps[:w, :], act_tile[:, k:k + 2, 0, r0:r0 + w],
                                 w2l_slab[:, k:k + 2, :],
                                 start=False, stop=True, perf_mode=DR)
            ysb = outp.tile([128, 512], BF16, tag="ysb",
                            name=f"ysb_{b['tag']}_{hc}_{tt}")
            if b["cw"] is not None:
                nc.scalar.activation(
                    out=ysb[:w, :], in_=ps[:w, :], func=AF.Copy,
                    scale=b["cw"][:w, tt:tt + 1])
            else:
                nc.scalar.activation(out=ysb[:w, :], in_=ps[:w, :],
                                     func=AF.Copy, scale=S2_EVICT)
            nc.sync.dma_start(
                out=b["out"][b["row0"] + r0: b["row0"] + r0 + w,
                             hc * 512:(hc + 1) * 512],
                in_=ysb[:w, :])


def _build(cap):
    nc = bacc.Bacc("TRN2", target_bir_lowering=False, debug=False)

    cwcols = (cap + 127) // 128
    aps = {}
    for j in range(2):
        aps[f"xs{j}"] = nc.dram_tensor(f"xs{j}", [H, 2, cap], F8,
                                       kind="ExternalInput").ap()
        aps[f"w1h{j}"] = nc.dram_tensor(f"w1h{j}", [H, I2], F8,
                                        kind="ExternalInput").ap()
        aps[f"w1l{j}"] = nc.dram_tensor(f"w1l{j}", [H, I2], F8,
                                        kind="ExternalInput").ap()
        aps[f"w2h{j}"] = nc.dram_tensor(f"w2h{j}", [1536, H], F8,
                                        kind="ExternalInput").ap()
        aps[f"w2l{j}"] = nc.dram_tensor(f"w2l{j}", [1536, H], F8,
                                        kind="ExternalInput").ap()
        aps[f"cw{j}"] = nc.dram_tensor(f"cw{j}", [cwcols * 128], F32,
                                       kind="ExternalInput").ap()
        aps[f"y{j}"] = nc.dram_tensor(f"y{j}", [cap, H], BF16,
                                      kind="ExternalOutput").ap()
    aps["xt"] = nc.dram_tensor("xt", [H, 2, T], F8, kind="ExternalInput").ap()
    aps["sw1h"] = nc.dram_tensor("sw1h", [H, 2 * SSLP], F8,
                                 kind="ExternalInput").ap()
    aps["sw1l"] = nc.dram_tensor("sw1l", [H, 2 * SSLP], F8,
                                 kind="ExternalInput").ap()
    aps["sw2h"] = nc.dram_tensor("sw2h", [512, H], F8,
                                 kind="ExternalInput").ap()
    aps["sw2l"] = nc.dram_tensor("sw2l", [512, H], F8,
                                 kind="ExternalInput").ap()
    aps["ys"] = nc.dram_tensor("ys", [T, H], BF16, kind="ExternalOutput").ap()

    import contextlib
    with tile.TileContext(nc) as tc, contextlib.ExitStack() as ctx:
        pools = {
            "x": ctx.enter_context(tc.tile_pool(name="x", bufs=2)),
            "w1": ctx.enter_context(tc.tile_pool(name="w1", bufs=8)),
            "w2": ctx.enter_context(tc.tile_pool(name="w2", bufs=4)),
            "act": ctx.enter_context(tc.tile_pool(name="act", bufs=2)),
            "stage": ctx.enter_context(tc.tile_pool(name="stage", bufs=4)),
            "out": ctx.enter_context(tc.tile_pool(name="out", bufs=6)),
            # separate s1/s2 PSUM pools: the cross-block s2 deferral must
            # never be starved of PSUM slots by the next block's stalled s1
            "ps": ctx.enter_context(tc.tile_pool(name="ps", bufs=4,
                                                 space="PSUM")),
            "ps2": ctx.enter_context(tc.tile_pool(name="ps2", bufs=4,
                                                  space="PSUM")),
            "misc": ctx.enter_context(tc.tile_pool(name="misc", bufs=2)),
        }
        pools["tc"] = tc
        cw_tiles = {}

        def get_cw(j):
            if j not in cw_tiles:
                cw_r = aps[f"cw{j}"].rearrange("(n p) -> p n", p=128)
                cw_tiles[j] = pools["misc"].tile([128, cwcols], F32,
                                                 tag=f"cw{j}", name=f"cw{j}_t")
                nc.sync.dma_start(out=cw_tiles[j][:], in_=cw_r[:])
            return cw_tiles[j]

        shared_res = [None, None, None, None]
        blocks = []
        for j in range(2):
            blocks.append(dict(
                tag=f"e{j}", n_go=11, act_kt=12, odd_kt=True, ntok=cap,
                x_ap=aps[f"xs{j}"], x_off=0,
                w1h=aps[f"w1h{j}"], w1l=aps[f"w1l{j}"],
                w2h=aps[f"w2h{j}"], w2l=aps[f"w2l{j}"],
                out=aps[f"y{j}"], row0=0, cw_j=j, w2_res=None,
                hipri_slab=True,
            ))
        for half in range(2):
            blocks.append(dict(
                tag=f"sh{half}", n_go=3, act_kt=4, odd_kt=True, ntok=1024,
                x_ap=aps["xt"], x_off=half * 1024,
                w1h=aps["sw1h"], w1l=aps["sw1l"],
                w2h=aps["sw2h"], w2l=aps["sw2l"],
                out=aps["ys"], row0=half * 1024, cw_j=None,
                w2_res=shared_res, hipri_slab=True,
            ))

        def s1_chunks(n):
            b = blocks[n]
            return _nchunks(b["ntok"], first=256 if n == 0 else None)

        def load_x(n, hipri_first_only=False):
            b = blocks[n]
            x_r = b["x_ap"].rearrange("(k p) s t -> p k s t", p=128)
            xt_tile = pools["x"].tile([128, 16, 2, b["ntok"]], F8, tag="xsel",
                                      name=f"x_{b['tag']}")
            t0 = 0
            for ci, w in enumerate(s1_chunks(n)):
                for s in range(2):  # per-slot DMA keeps APs 3-D (balancer cap)
                    src = x_r[:, :, s, b["x_off"] + t0: b["x_off"] + t0 + w]
                    dst = xt_tile[:, :, s, t0:t0 + w]
                    if ci == 0 or not hipri_first_only:
                        with tc.high_priority():
                            nc.sync.dma_start(out=dst, in_=src)
                    else:
                        nc.sync.dma_start(out=dst, in_=src)
                t0 += w
            return xt_tile

        def emit_s2_part(b, act_tile, part):
            b2 = dict(b)
            b2["cw"] = None if b["cw_j"] is None else get_cw(b["cw_j"])
            _emit_s2(nc, pools, b=b2, act_tile=act_tile, part=part)

        # Emit s1(n), then block n+1's x-load, then the previous block's
        # deferred s2 half, then s2(n) part 1: the next x-load overlaps s2(n)
        # compute instead of queueing behind its weight slabs.
        x_tiles = [load_x(0, hipri_first_only=True)]
        deferred = None
        for n, b in enumerate(blocks):
            act_tile = pools["act"].tile([128, b["act_kt"], 2, b["ntok"]], F8,
                                         tag="act", name=f"act_{b['tag']}")
            # zero the pad k-tile (hi slot is read by the odd-k leftover
            # instruction; lo slot never read)
            nc.gpsimd.memset(act_tile[:, b["act_kt"] - 1, 0, :], 0.0)
            _emit_s1(nc, pools, b=b, x_tile=x_tiles[n], act_tile=act_tile,
                     chunks=s1_chunks(n))
            if n + 1 < len(blocks):
                x_tiles.append(load_x(n + 1))
            if deferred is not None:
                emit_s2_part(*deferred, part=2)
                deferred = None
            emit_s2_part(b, act_tile, part=1)
            deferred = (b, act_tile)
        if deferred is not None:
            emit_s2_part(*deferred, part=2)

    nc.compile()
    return nc


def _route(xf, gate_w):
    """Host router: fp32 softmax + top-6, matching jax bitwise when possible."""
    try:
        import jax
        import jax.numpy as jnp

        cpu = jax.devices("cpu")[0]
        with jax.default_device(cpu):
            logits = jnp.asarray(xf) @ jnp.asarray(gate_w).T
            probs = jax.nn.softmax(logits.astype(jnp.float32), axis=-1)
            _, sel = jax.lax.top_k(probs, TOPK)
        return np.asarray(probs), np.asarray(sel)
    except Exception:
        logits = xf @ gate_w.T
        m = logits.max(axis=-1, keepdims=True)
        e = np.exp(logits - m, dtype=np.float32)
        probs = e / e.sum(axis=-1, keepdims=True)
        sel = np.argsort(-probs, axis=-1, kind="stable")[:, :TOPK]
        return probs, sel


def _split8(a, s):
    """-> (hi, lo) fp8 arrays with a*s ~= hi + lo."""
    sa = (a * s).astype(np.float32)
    hi = sa.astype(F8NP)
    lo = (sa - hi.astype(np.float32)).astype(F8NP)
    return hi, lo


def _interleave_hl(hi, lo):
    """[R, C] pair -> [R, 2, C] fp8."""
    out = np.empty((hi.shape[0], 2, hi.shape[1]), F8NP)
    out[:, 0, :] = hi
    out[:, 1, :] = lo
    return out


def kernel(x, gate_w, w1, w2, shared_w1, shared_w2):
    x = np.asarray(x, np.float32)
    gate_w = np.asarray(gate_w, np.float32)
    w1 = np.asarray(w1, np.float32)
    w2 = np.asarray(w2, np.float32)
    shared_w1 = np.asarray(shared_w1, np.float32)
    shared_w2 = np.asarray(shared_w2, np.float32)

    B, S, Hd = x.shape
    xf = np.ascontiguousarray(x.reshape(-1, Hd))  # [T, H]

    probs, sel = _route(xf, gate_w)
    onehot = np.zeros((T, E), bool)
    onehot[np.arange(T)[:, None], sel] = True
    idx_e = [np.nonzero(onehot[:, e])[0] for e in range(E)]
    counts = np.array([len(ix) for ix in idx_e])

    cap = CAP0
    while counts.max() > cap:
        cap += 64
    if cap not in _compiled:
        _compiled[cap] = _build(cap)
    nc = _compiled[cap]

    # quantize x once: [T, H] hi/lo
    xq_hi, xq_lo = _split8(xf, SX)
    xt2 = np.empty((H, 2, T), F8NP)
    xt2[:, 0, :] = xq_hi.T
    xt2[:, 1, :] = xq_lo.T

    cwcols = (cap + 127) // 128
    in_maps = []
    for c in range(NCORES):
        m = {"xt": xt2}
        for j in range(2):
            e = 2 * c + j
            ix = idx_e[e]
            xs2 = np.zeros((H, 2, cap), F8NP)
            xs2[:, 0, :len(ix)] = xq_hi[ix].T
            xs2[:, 1, :len(ix)] = xq_lo[ix].T
            m[f"xs{j}"] = xs2
            hi, lo = _split8(w1[e].T, SW1)  # [H, I2]
            m[f"w1h{j}"] = hi
            m[f"w1l{j}"] = lo
            hi, lo = _split8(w2[e].T, SW2)  # [I, H]
            w2h = np.zeros((1536, H), F8NP)
            w2l = np.zeros((1536, H), F8NP)
            w2h[:I] = hi
            w2l[:I] = lo
            w2h[I:] = hi[-128:]  # dup of k-tile 10 for the odd-k leftover
            m[f"w2h{j}"] = w2h
            m[f"w2l{j}"] = w2l
            cw = np.zeros(cwcols * 128, np.float32)
            cw[: len(ix)] = probs[ix, e] * S2_EVICT
            m[f"cw{j}"] = cw
        sl = slice(SSL * c, SSL * (c + 1))
        sg = np.zeros((H, SSLP), np.float32)
        su = np.zeros((H, SSLP), np.float32)
        sg[:, :SSL] = shared_w1[sl].T
        su[:, :SSL] = shared_w1[ISH + SSL * c: ISH + SSL * (c + 1)].T
        hi_g, lo_g = _split8(sg, SW1)
        hi_u, lo_u = _split8(su, SW1)
        m["sw1h"] = np.concatenate([hi_g, hi_u], axis=1)
        m["sw1l"] = np.concatenate([lo_g, lo_u], axis=1)
        s2w = np.zeros((512, H), np.float32)
        s2w[:SSL] = shared_w2[:, sl].T
        hi, lo = _split8(s2w, SW2)
        hi[SSLP:] = hi[SSLP - 128: SSLP]  # dup k-tile 2
        lo[SSLP:] = 0
        m["sw2h"] = hi
        m["sw2l"] = lo
        in_maps.append(m)

    try:
        res = run_bass_kernel_spmd(nc, in_maps, list(range(NCORES)))
    except ModuleNotFoundError:
        os.environ["BASS_NEVER_TRACE"] = "1"
        res = run_bass_kernel_spmd(nc, in_maps, list(range(NCORES)))
    global last_result
    last_result = res

    out = np.zeros((T, H), np.float32)
    for c in range(NCORES):
        out += res.results[c]["ys"].astype(np.float32)
        for j in range(2):
            e = 2 * c + j
            ix = idx_e[e]
            out[ix] += res.results[c][f"y{j}"][: len(ix)].astype(np.float32)

    return out.reshape(B, S, Hd)


# revision 25
# speedup vs baseline: 1.6449x; 1.2145x over previous
"""DeepseekMoE layer on 8 TRN2 NeuronCores — expert-parallel Bass/Tile kernel.

Strategy (self-contained, shapes hardcoded for this problem):
  H=2048, T=2048 tokens, E=16 experts, top-6, I=1408, shared IS=2816.

  Sharding (done on host inside kernel(), per the full-input contract):
    - Router (softmax + top-6) computed on host in fp32 (jax-on-CPU when
      available so near-tie selections match the jax reference bitwise)
      -> per-expert token lists (the "all-to-all dispatch" decision).
    - Core c owns experts 2c, 2c+1 (capacity-padded to CAP tokens each);
      shared expert sharded over its intermediate dim (352 rows per core,
      padded to 384 = 3*128).
    - Each core returns per-expert outputs [CAP, H] bf16 (pre-scaled by
      routing weights) and a dense shared partial [T, H] bf16; the host
      scatter-adds in fp32.

  Arithmetic: all matmuls run as fp8(e4m3) DoubleRow pairs at 0.5 cyc/row,
  using a hi+lo residual decomposition of every operand:
      a*s ~= a_hi + a_lo   (a_hi = fp8(a*s), a_lo = fp8(a*s - a_hi))
      a*b*s_a*s_b ~= a_hi*b_hi + a_lo*b_hi + a_hi*b_lo     (lo*lo dropped)
  Three DoubleRow instructions per k-tile pair = 0.75 cyc/row/k-tile, a
  1.33x speedup over bf16/fp32r with ~bf16 effective precision (measured
  end-to-end rel err 2.6e-3 vs the 2e-2 gate).

  Scales: x*4, w1*64, w2*128. Stage-1 PSUM: gate = g*256 (Silu evicted with
  scale 2^-8); up = u*256, fused DVE (ps_u * 1/16) * silu_g = act*16, which
  is split hi/lo to fp8 for stage 2. Stage-2 PSUM = y*2048; eviction scale
  folds 2^-11 into the per-token routing weight (or a constant for the
  shared expert). Odd k-tile counts (11 expert, 3 shared) are handled by
  host-side slab rows: hi-slab gets a duplicated last k-tile, lo-slab gets
  zeros, plus a one-time memset of the act tile's pad k-tile.
"""

import os
import sys

sys.path.insert(0, "/opt/trn_rl_repo")

import numpy as np
import ml_dtypes

import concourse.bass as bass  # noqa: F401
import concourse.tile as tile
from concourse import bacc, mybir
from concourse.bass_utils import run_bass_kernel_spmd

H = 2048
T = 2048
E = 16
TOPK = 6
I2 = 2816  # 2*I
I = 1408
ISH = 2816  # shared intermediate (per gate/up half)
NCORES = 8
CAP0 = 832  # per-expert token capacity; grown in 64s if exceeded
SSL = 352  # shared-intermediate slice per core
SSLP = 384  # padded to 3*128

SX, SW1, SW2, SACT = 4.0, 64.0, 128.0, 16.0
S1_EVICT = 1.0 / (SW1 * SX)  # 2^-8
S1_UP = SACT / (SW1 * SX)  # 1/16
S2_EVICT = 1.0 / (SACT * SW2)  # 2^-11

F8NP = ml_dtypes.float8_e4m3
BF16NP = ml_dtypes.bfloat16
F32 = mybir.dt.float32
F8 = mybir.dt.float8e4
BF16 = mybir.dt.bfloat16
AF = mybir.ActivationFunctionType
ALU = mybir.AluOpType
DR = mybir.MatmulPerfMode.DoubleRow

_compiled = {}
last_result = None  # BassKernelResults of the most recent run (for profiling)


def _nchunks(n, first=None):
    """Split n into <=512 free-dim chunks; optional smaller first chunk so the
    first PSUM group starts after a fraction of the x block has landed."""
    out = [first] if first else []
    n -= first or 0
    while n > 0:
        w = min(512, n)
        out.append(w)
        n -= w
    return out


def _emit_mm3(nc, ps, wh_slab, wl_slab, x_tile, xc0, w, n_kt, first, last,
              use_wlo=True):
    """Accumulate sum_k w~[k].T @ x~[k] into ps[:, :w] via 3-product fp8
    DoubleRow instructions. wh/wl slabs: [128, n_kt(+pad), 128]; x_tile:
    [128, n_kt, 2, tokens] (hi/lo interleaved), token cols [xc0, xc0+w).
    n_kt may be odd: slabs carry a dup/zero pad k-tile (see module doc).
    use_wlo=False drops the w-residual product (2-product mode)."""
    xs = x_tile[:, :, :, xc0:xc0 + w]
    npair = n_kt // 2
    per = 3 if use_wlo else 2
    n3 = npair * per + (2 if n_kt % 2 else 0)
    i = 0
    for kp in range(npair):
        k = 2 * kp
        prods = [
            (wh_slab[:, k:k + 2, :], xs[:, k:k + 2, 0, :]),
            (wh_slab[:, k:k + 2, :], xs[:, k:k + 2, 1, :]),
        ]
        if use_wlo:
            prods.append((wl_slab[:, k:k + 2, :], xs[:, k:k + 2, 0, :]))
        for lhsT, rhs in prods:
            nc.tensor.matmul(ps[:, :w], lhsT, rhs,
                             start=(first and i == 0),
                             stop=(last and i == n3 - 1), perf_mode=DR)
            i += 1
    if n_kt % 2:
        k = n_kt - 1
        # (w_hi[k], w_hi[k]-dup) x (x_hi[k], x_lo[k])
        nc.tensor.matmul(ps[:, :w], wh_slab[:, k:k + 2, :], xs[:, k, :, :],
                         start=(first and i == 0), stop=False, perf_mode=DR)
        # (w_lo[k], zeros) x (x_hi[k], x_hi[k+1]-garbage*0)
        nc.tensor.matmul(ps[:, :w], wl_slab[:, k:k + 2, :],
                         xs[:, k:k + 2, 0, :],
                         start=False, stop=last, perf_mode=DR)


def _emit_s1(nc, pools, *, b, x_tile, act_tile, chunks, ogroup=None):
    """Stage 1: per gate-o-tile, compute ps_g/ps_u via 3-product DoubleRow,
    then evict: ACT Silu -> ag; DVE (ps_u/16)*ag -> stage; ACT cast -> act_hi;
    DVE stage-hi -> act_lo. act_tile: [128, act_kt, 2, ntok] fp8.

    ogroup: if set, loop chunk-outer within o-groups of that size so the PE
    rides the incoming x stream instead of stalling o-by-o (startup block)."""
    w1p, psp, stp = pools["w1"], pools["ps"], pools["stage"]
    tc = pools["tc"]
    n_go = b["n_go"]
    w1h_r = b["w1h"].rearrange("(k p) o -> p k o", p=128)
    w1l_r = b["w1l"].rearrange("(k p) o -> p k o", p=128)
    spans = []
    t0 = 0
    for w in chunks:
        spans.append((t0, w))
        t0 += w

    g_wlo = not b.get("drop_gwlo")

    def load_slabs(o, hipri):
        slabs = []
        for gate, ot in ((True, o), (False, n_go + o)):
            for si, src in enumerate((w1h_r, w1l_r)):
                if si == 1 and gate and not g_wlo:
                    slabs.append(None)
                    continue
                sl = w1p.tile([128, 16, 128], F8, tag="w1slab",
                              name=f"w1_{b['tag']}_{ot}_{si}")
                if hipri:
                    with tc.high_priority():
                        nc.sync.dma_start(
                            out=sl[:], in_=src[:, :, ot * 128:(ot + 1) * 128])
                else:
                    nc.sync.dma_start(
                        out=sl[:], in_=src[:, :, ot * 128:(ot + 1) * 128])
                slabs.append(sl)
        return slabs

    def emit_o_chunk(o, slabs, ci, t0, w):
        gwh, gwl, uwh, uwl = slabs
        ps_g = psp.tile([128, 512], F32, tag="ps", name=f"psg_{o}_{ci}")
        ps_u = psp.tile([128, 512], F32, tag="ps", name=f"psu_{o}_{ci}")
        _emit_mm3(nc, ps_g, gwh, gwl, x_tile, t0, w, 16, True, True,
                  use_wlo=g_wlo)
        _emit_mm3(nc, ps_u, uwh, uwl, x_tile, t0, w, 16, True, True)
        ag = stp.tile([128, 512], F32, tag="stage", name=f"ag_{o}_{ci}")
        st = stp.tile([128, 512], F32, tag="stage", name=f"st_{o}_{ci}")
        nc.scalar.activation(out=ag[:, :w], in_=ps_g[:, :w], func=AF.Silu,
                             scale=S1_EVICT)
        nc.vector.scalar_tensor_tensor(
            out=st[:, :w], in0=ps_u[:, :w], scalar=S1_UP, in1=ag[:, :w],
            op0=ALU.mult, op1=ALU.mult)
        hi = act_tile[:, o, 0, t0:t0 + w]
        nc.scalar.activation(out=hi, in_=st[:, :w], func=AF.Copy)
        nc.vector.tensor_sub(act_tile[:, o, 1, t0:t0 + w], st[:, :w], hi)

    if ogroup is None:
        for o in range(n_go):
            slabs = load_slabs(o, o == 0 and b.get("hipri_slab"))
            for ci, (t0, w) in enumerate(spans):
                emit_o_chunk(o, slabs, ci, t0, w)
    else:
        for g0 in range(0, n_go, ogroup):
            os_ = range(g0, min(g0 + ogroup, n_go))
            slabs = {o: load_slabs(o, g0 == 0 and b.get("hipri_slab"))
                     for o in os_}
            for ci, (t0, w) in enumerate(spans):
                for o in os_:
                    emit_o_chunk(o, slabs[o], ci, t0, w)


def _emit_s2(nc, pools, *, b, act_tile, part):
    """Stage 2: out[t, hc] = sum_k act~[k].T @ w2~[k], 3-product DoubleRow
    with act (hi/lo) stationary and w2 slabs moving. Evict with per-token
    (expert) or constant (shared) scale to bf16, DMA out."""
    w2p, psp, outp = pools["w2"], pools["ps2"], pools["out"]
    n_kt = b["act_kt"] - 1 if b["odd_kt"] else b["act_kt"]
    ntok = b["ntok"]
    use_wlo = not b.get("drop_s2wlo")
    w2h_r = b["w2h"].rearrange("(k p) h -> p k h", p=128)
    w2l_r = b["w2l"].rearrange("(k p) h -> p k h", p=128)
    hc_list = {1: range(2), 2: range(2, 4)}[part]
    ntt = (ntok + 127) // 128
    for hc in hc_list:
        if b.get("w2_res") is not None:
            if b["w2_res"][hc] is None:
                sh = w2p.tile([128, b["act_kt"], 512], F8, tag="w2res",
                              bufs=8, name=f"w2h_res_{hc}")
                sl = w2p.tile([128, b["act_kt"], 512], F8, tag="w2res",
                              bufs=8, name=f"w2l_res_{hc}")
                nc.sync.dma_start(out=sh[:],
                                  in_=w2h_r[:, :, hc * 512:(hc + 1) * 512])
                nc.sync.dma_start(out=sl[:],
                                  in_=w2l_r[:, :, hc * 512:(hc + 1) * 512])
                b["w2_res"][hc] = (sh, sl)
            w2h_slab, w2l_slab = b["w2_res"][hc]
        else:
            w2h_slab = w2p.tile([128, b["act_kt"], 512], F8, tag="w2slab",
                                name=f"w2h_{b['tag']}_{hc}")
            nc.sync.dma_start(out=w2h_slab[:],
                              in_=w2h_r[:, :, hc * 512:(hc + 1) * 512])
            w2l_slab = None
            if use_wlo:
                w2l_slab = w2p.tile([128, b["act_kt"], 512], F8, tag="w2slab",
                                    name=f"w2l_{b['tag']}_{hc}")
                nc.sync.dma_start(out=w2l_slab[:],
                                  in_=w2l_r[:, :, hc * 512:(hc + 1) * 512])
        for tt in range(ntt):
            r0 = tt * 128
            w = min(128, ntok - r0)
            ps = psp.tile([128, 512], F32, tag="ps2", name=f"ps2_{hc}_{tt}")
            npair = n_kt // 2
            per = 3 if use_wlo else 2
            n3 = npair * per + (n_kt % 2) * (2 if use_wlo else 1)
            i = 0
            for kp in range(npair):
                k = 2 * kp
                prods = [
                    (act_tile[:, k:k + 2, 0, r0:r0 + w], w2h_slab[:, k:k + 2, :]),
                    (act_tile[:, k:k + 2, 1, r0:r0 + w], w2h_slab[:, k:k + 2, :]),
                ]
                if use_wlo:
                    prods.append((act_tile[:, k:k + 2, 0, r0:r0 + w],
                                  w2l_slab[:, k:k + 2, :]))
                for lhsT, rhs in prods:
                    nc.tensor.matmul(ps[:w, :], lhsT, rhs, start=(i == 0),
                                     stop=(i == n3 - 1), perf_mode=DR)
                    i += 1
            if n_kt % 2:
                k = n_kt - 1
                nc.tensor.matmul(ps[:w, :], act_tile[:, k, :, r0:r0 + w],
                                 w2h_slab[:, k:k + 2, :], start=False,
                                 stop=(not use_wlo), perf_mode=DR)
                if use_wlo:
                    nc.tensor.matmul(ps[:w, :],
                                     act_tile[:, k:k + 2, 0, r0:r0 + w],
                                     w2l_slab[:, k:k + 2, :],
                                     start=False, stop=True, perf_mode=DR)
            ysb = outp.tile([128, 512], BF16, tag="ysb",
                            name=f"ysb_{b['tag']}_{hc}_{tt}")
            if b["cw"] is not None:
                nc.scalar.activation(
                    out=ysb[:w, :], in_=ps[:w, :], func=AF.Copy,
                    scale=b["cw"][:w, tt:tt + 1])
            else:
                nc.scalar.activation(out=ysb[:w, :], in_=ps[:w, :],
                                     func=AF.Copy, scale=S2_EVICT)
            nc.sync.dma_start(
                out=b["out"][b["row0"] + r0: b["row0"] + r0 + w,
                             hc * 512:(hc + 1) * 512],
                in_=ysb[:w, :])


def _build(cap, order=(0, 1, 2, 3), first_chunk=256, defer_parts=(2,),
           ogroup0=None, w1bufs=8, drop_gwlo=False, drop_s2wlo=False):
    nc = bacc.Bacc("TRN2", target_bir_lowering=False, debug=False)

    cwcols = (cap + 127) // 128
    aps = {}
    for j in range(2):
        aps[f"xs{j}"] = nc.dram_tensor(f"xs{j}", [H, 2, cap], F8,
                                       kind="ExternalInput").ap()
        aps[f"w1h{j}"] = nc.dram_tensor(f"w1h{j}", [H, I2], F8,
                                        kind="ExternalInput").ap()
        aps[f"w1l{j}"] = nc.dram_tensor(f"w1l{j}", [H, I2], F8,
                                        kind="ExternalInput").ap()
        aps[f"w2h{j}"] = nc.dram_tensor(f"w2h{j}", [1536, H], F8,
                                        kind="ExternalInput").ap()
        aps[f"w2l{j}"] = nc.dram_tensor(f"w2l{j}", [1536, H], F8,
                                        kind="ExternalInput").ap()
        aps[f"cw{j}"] = nc.dram_tensor(f"cw{j}", [cwcols * 128], F32,
                                       kind="ExternalInput").ap()
        aps[f"y{j}"] = nc.dram_tensor(f"y{j}", [cap, H], BF16,
                                      kind="ExternalOutput").ap()
    aps["xt"] = nc.dram_tensor("xt", [H, 2, T], F8, kind="ExternalInput").ap()
    aps["sw1h"] = nc.dram_tensor("sw1h", [H, 2 * SSLP], F8,
                                 kind="ExternalInput").ap()
    aps["sw1l"] = nc.dram_tensor("sw1l", [H, 2 * SSLP], F8,
                                 kind="ExternalInput").ap()
    aps["sw2h"] = nc.dram_tensor("sw2h", [512, H], F8,
                                 kind="ExternalInput").ap()
    aps["sw2l"] = nc.dram_tensor("sw2l", [512, H], F8,
                                 kind="ExternalInput").ap()
    aps["ys"] = nc.dram_tensor("ys", [T, H], BF16, kind="ExternalOutput").ap()

    import contextlib
    with tile.TileContext(nc) as tc, contextlib.ExitStack() as ctx:
        pools = {
            "x": ctx.enter_context(tc.tile_pool(name="x", bufs=2)),
            "w1": ctx.enter_context(tc.tile_pool(name="w1", bufs=w1bufs)),
            "w2": ctx.enter_context(tc.tile_pool(name="w2", bufs=4)),
            "act": ctx.enter_context(tc.tile_pool(name="act", bufs=2)),
            "stage": ctx.enter_context(tc.tile_pool(name="stage", bufs=4)),
            "out": ctx.enter_context(tc.tile_pool(name="out", bufs=6)),
            # separate s1/s2 PSUM pools: the cross-block s2 deferral must
            # never be starved of PSUM slots by the next block's stalled s1
            "ps": ctx.enter_context(tc.tile_pool(name="ps", bufs=4,
                                                 space="PSUM")),
            "ps2": ctx.enter_context(tc.tile_pool(name="ps2", bufs=4,
                                                  space="PSUM")),
            "misc": ctx.enter_context(tc.tile_pool(name="misc", bufs=2)),
        }
        pools["tc"] = tc
        cw_tiles = {}

        def get_cw(j):
            if j not in cw_tiles:
                cw_r = aps[f"cw{j}"].rearrange("(n p) -> p n", p=128)
                cw_tiles[j] = pools["misc"].tile([128, cwcols], F32,
                                                 tag=f"cw{j}", name=f"cw{j}_t")
                nc.sync.dma_start(out=cw_tiles[j][:], in_=cw_r[:])
            return cw_tiles[j]

        shared_res = [None, None, None, None]
        all_blocks = []
        for j in range(2):
            all_blocks.append(dict(
                tag=f"e{j}", n_go=11, act_kt=12, odd_kt=True, ntok=cap,
                x_ap=aps[f"xs{j}"], x_off=0,
                w1h=aps[f"w1h{j}"], w1l=aps[f"w1l{j}"],
                w2h=aps[f"w2h{j}"], w2l=aps[f"w2l{j}"],
                out=aps[f"y{j}"], row0=0, cw_j=j, w2_res=None,
                hipri_slab=True, drop_gwlo=drop_gwlo, drop_s2wlo=drop_s2wlo,
            ))
        for half in range(2):
            all_blocks.append(dict(
                tag=f"sh{half}", n_go=3, act_kt=4, odd_kt=True, ntok=1024,
                x_ap=aps["xt"], x_off=half * 1024,
                w1h=aps["sw1h"], w1l=aps["sw1l"],
                w2h=aps["sw2h"], w2l=aps["sw2l"],
                out=aps["ys"], row0=half * 1024, cw_j=None,
                w2_res=shared_res, hipri_slab=True,
            ))

        blocks = [all_blocks[i] for i in order]

        def s1_chunks(n):
            b = blocks[n]
            return _nchunks(b["ntok"], first=first_chunk if n == 0 else None)

        def load_x(n, hipri_first_only=False):
            b = blocks[n]
            x_r = b["x_ap"].rearrange("(k p) s t -> p k s t", p=128)
            xt_tile = pools["x"].tile([128, 16, 2, b["ntok"]], F8, tag="xsel",
                                      name=f"x_{b['tag']}")
            t0 = 0
            for ci, w in enumerate(s1_chunks(n)):
                for s in range(2):  # per-slot DMA keeps APs 3-D (balancer cap)
                    src = x_r[:, :, s, b["x_off"] + t0: b["x_off"] + t0 + w]
                    dst = xt_tile[:, :, s, t0:t0 + w]
                    if ci == 0 or not hipri_first_only:
                        with tc.high_priority():
                            nc.sync.dma_start(out=dst, in_=src)
                    else:
                        nc.sync.dma_start(out=dst, in_=src)
                t0 += w
            return xt_tile

        def emit_s2_part(b, act_tile, part):
            b2 = dict(b)
            b2["cw"] = None if b["cw_j"] is None else get_cw(b["cw_j"])
            _emit_s2(nc, pools, b=b2, act_tile=act_tile, part=part)

        # Emit s1(n), then block n+1's x-load, then the previous block's
        # deferred s2 half, then s2(n) part 1: the next x-load overlaps s2(n)
        # compute instead of queueing behind its weight slabs.
        x_tiles = [load_x(0, hipri_first_only=True)]
        deferred = None
        for n, b in enumerate(blocks):
            act_tile = pools["act"].tile([128, b["act_kt"], 2, b["ntok"]], F8,
                                         tag="act", name=f"act_{b['tag']}")
            # zero the pad k-tile (hi slot is read by the odd-k leftover
            # instruction; lo slot never read)
            nc.gpsimd.memset(act_tile[:, b["act_kt"] - 1, 0, :], 0.0)
            _emit_s1(nc, pools, b=b, x_tile=x_tiles[n], act_tile=act_tile,
                     chunks=s1_chunks(n), ogroup=ogroup0 if n == 0 else None)
            if n + 1 < len(blocks):
                x_tiles.append(load_x(n + 1))
            if deferred is not None:
                for p in defer_parts:
                    emit_s2_part(*deferred, part=p)
                deferred = None
            for p in (1, 2):
                if p not in defer_parts:
                    emit_s2_part(b, act_tile, part=p)
            deferred = (b, act_tile)
        if deferred is not None:
            for p in defer_parts:
                emit_s2_part(*deferred, part=p)

    nc.compile()
    return nc


def _route(xf, gate_w):
    """Host router: fp32 softmax + top-6, matching jax bitwise when possible."""
    try:
        import jax
        import jax.numpy as jnp

        cpu = jax.devices("cpu")[0]
        with jax.default_device(cpu):
            logits = jnp.asarray(xf) @ jnp.asarray(gate_w).T
            probs = jax.nn.softmax(logits.astype(jnp.float32), axis=-1)
            _, sel = jax.lax.top_k(probs, TOPK)
        return np.asarray(probs), np.asarray(sel)
    except Exception:
        logits = xf @ gate_w.T
        m = logits.max(axis=-1, keepdims=True)
        e = np.exp(logits - m, dtype=np.float32)
        probs = e / e.sum(axis=-1, keepdims=True)
        sel = np.argsort(-probs, axis=-1, kind="stable")[:, :TOPK]
        return probs, sel


def _split8(a, s):
    """-> (hi, lo) fp8 arrays with a*s ~= hi + lo."""
    sa = (a * s).astype(np.float32)
    hi = sa.astype(F8NP)
    lo = (sa - hi.astype(np.float32)).astype(F8NP)
    return hi, lo


def _interleave_hl(hi, lo):
    """[R, C] pair -> [R, 2, C] fp8."""
    out = np.empty((hi.shape[0], 2, hi.shape[1]), F8NP)
    out[:, 0, :] = hi
    out[:, 1, :] = lo
    return out


def kernel(x, gate_w, w1, w2, shared_w1, shared_w2):
    x = np.asarray(x, np.float32)
    gate_w = np.asarray(gate_w, np.float32)
    w1 = np.asarray(w1, np.float32)
    w2 = np.asarray(w2, np.float32)
    shared_w1 = np.asarray(shared_w1, np.float32)
    shared_w2 = np.asarray(shared_w2, np.float32)

    B, S, Hd = x.shape
    xf = np.ascontiguousarray(x.reshape(-1, Hd))  # [T, H]

    probs, sel = _route(xf, gate_w)
    onehot = np.zeros((T, E), bool)
    onehot[np.arange(T)[:, None], sel] = True
    idx_e = [np.nonzero(onehot[:, e])[0] for e in range(E)]
    counts = np.array([len(ix) for ix in idx_e])

    cap = CAP0
    while counts.max() > cap:
        cap += 64
    if cap not in _compiled:
        _compiled[cap] = _build(cap, order=(0, 2, 3, 1), defer_parts=(),
                                drop_gwlo=True, drop_s2wlo=True)
    nc = _compiled[cap]

    # quantize x once: [T, H] hi/lo
    xq_hi, xq_lo = _split8(xf, SX)
    xt2 = np.empty((H, 2, T), F8NP)
    xt2[:, 0, :] = xq_hi.T
    xt2[:, 1, :] = xq_lo.T

    cwcols = (cap + 127) // 128
    in_maps = []
    for c in range(NCORES):
        m = {"xt": xt2}
        for j in range(2):
            e = 2 * c + j
            ix = idx_e[e]
            xs2 = np.zeros((H, 2, cap), F8NP)
            xs2[:, 0, :len(ix)] = xq_hi[ix].T
            xs2[:, 1, :len(ix)] = xq_lo[ix].T
            m[f"xs{j}"] = xs2
            hi, lo = _split8(w1[e].T, SW1)  # [H, I2]
            m[f"w1h{j}"] = hi
            m[f"w1l{j}"] = lo
            hi, lo = _split8(w2[e].T, SW2)  # [I, H]
            w2h = np.zeros((1536, H), F8NP)
            w2l = np.zeros((1536, H), F8NP)
            w2h[:I] = hi
            w2l[:I] = lo
            w2h[I:] = hi[-128:]  # dup of k-tile 10 for the odd-k leftover
            m[f"w2h{j}"] = w2h
            m[f"w2l{j}"] = w2l
            cw = np.zeros(cwcols * 128, np.float32)
            cw[: len(ix)] = probs[ix, e] * S2_EVICT
            m[f"cw{j}"] = cw
        sl = slice(SSL * c, SSL * (c + 1))
        sg = np.zeros((H, SSLP), np.float32)
        su = np.zeros((H, SSLP), np.float32)
        sg[:, :SSL] = shared_w1[sl].T
        su[:, :SSL] = shared_w1[ISH + SSL * c: ISH + SSL * (c + 1)].T
        hi_g, lo_g = _split8(sg, SW1)
        hi_u, lo_u = _split8(su, SW1)
        m["sw1h"] = np.concatenate([hi_g, hi_u], axis=1)
        m["sw1l"] = np.concatenate([lo_g, lo_u], axis=1)
        s2w = np.zeros((512, H), np.float32)
        s2w[:SSL] = shared_w2[:, sl].T
        hi, lo = _split8(s2w, SW2)
        hi[SSLP:] = hi[SSLP - 128: SSLP]  # dup k-tile 2
        lo[SSLP:] = 0
        m["sw2h"] = hi
        m["sw2l"] = lo
        in_maps.append(m)

    try:
        res = run_bass_kernel_spmd(nc, in_maps, list(range(NCORES)))
    except ModuleNotFoundError:
        os.environ["BASS_NEVER_TRACE"] = "1"
        res = run_bass_kernel_spmd(nc, in_maps, list(range(NCORES)))
    global last_result
    last_result = res

    out = np.zeros((T, H), np.float32)
    for c in range(NCORES):
        out += res.results[c]["ys"].astype(np.float32)
        for j in range(2):
            e = 2 * c + j
            ix = idx_e[e]
            out[ix] += res.results[c][f"y{j}"][: len(ix)].astype(np.float32)

    return out.reshape(B, S, Hd)


# revision 49
# speedup vs baseline: 1.8253x; 1.1097x over previous
"""DeepseekMoE layer on 8 TRN2 NeuronCores — expert-parallel Bass/Tile kernel.

Strategy (self-contained, shapes hardcoded for this problem):
  H=2048, T=2048 tokens, E=16 experts, top-6, I=1408, shared IS=2816.

  Sharding (done on host inside kernel(), per the full-input contract):
    - Router (softmax + top-6) computed on host in fp32 (jax-on-CPU when
      available so near-tie selections match the jax reference bitwise)
      -> per-expert token lists (the "all-to-all dispatch" decision).
    - Core c owns experts 2c, 2c+1 (capacity-padded to CAP tokens each);
      shared expert sharded over its intermediate dim (352 rows per core,
      padded to 384 = 3*128).
    - Each core returns per-expert outputs [CAP, H] bf16 (pre-scaled by
      routing weights) and a dense shared partial [T, H] bf16; the host
      scatter-adds in fp32.

  Arithmetic: all matmuls run as fp8(e4m3) DoubleRow pairs at 0.5 cyc/row,
  using a hi+lo residual decomposition of every operand:
      a*s ~= a_hi + a_lo   (a_hi = fp8(a*s), a_lo = fp8(a*s - a_hi))
      a*b*s_a*s_b ~= a_hi*b_hi + a_lo*b_hi + a_hi*b_lo     (lo*lo dropped)
  Three DoubleRow instructions per k-tile pair = 0.75 cyc/row/k-tile, a
  1.33x speedup over bf16/fp32r with ~bf16 effective precision (measured
  end-to-end rel err 2.6e-3 vs the 2e-2 gate).

  Scales: x*4, w1*64, w2*128. Stage-1 PSUM: gate = g*256 (Silu evicted with
  scale 2^-8); up = u*256, fused DVE (ps_u * 1/16) * silu_g = act*16, which
  is split hi/lo to fp8 for stage 2. Stage-2 PSUM = y*2048; eviction scale
  folds 2^-11 into the per-token routing weight (or a constant for the
  shared expert). Odd k-tile counts (11 expert, 3 shared) are handled by
  host-side slab rows: hi-slab gets a duplicated last k-tile, lo-slab gets
  zeros, plus a one-time memset of the act tile's pad k-tile.
"""

import os
import sys

sys.path.insert(0, "/opt/trn_rl_repo")

import numpy as np
import ml_dtypes

import concourse.bass as bass  # noqa: F401
import concourse.tile as tile
from concourse import bacc, mybir
from concourse.bass_utils import run_bass_kernel_spmd

H = 2048
T = 2048
E = 16
TOPK = 6
I2 = 2816  # 2*I
I = 1408
ISH = 2816  # shared intermediate (per gate/up half)
NCORES = 8
CAP0 = 832  # per-expert token capacity; grown in 64s if exceeded
SSL = 352  # shared-intermediate slice per core
SSLP = 384  # padded to 3*128

SX, SW1, SW2, SACT = 4.0, 64.0, 128.0, 16.0
S1_EVICT = 1.0 / (SW1 * SX)  # 2^-8
S1_UP = SACT / (SW1 * SX)  # 1/16
S2_EVICT = 1.0 / (SACT * SW2)  # 2^-11

F8NP = ml_dtypes.float8_e4m3
BF16NP = ml_dtypes.bfloat16
F32 = mybir.dt.float32
F8 = mybir.dt.float8e4
BF16 = mybir.dt.bfloat16
AF = mybir.ActivationFunctionType
ALU = mybir.AluOpType
DR = mybir.MatmulPerfMode.DoubleRow

_compiled = {}
last_result = None  # BassKernelResults of the most recent run (for profiling)


def _nchunks(n, first=None):
    """Split n into <=512 free-dim chunks; optional smaller first chunk so the
    first PSUM group starts after a fraction of the x block has landed."""
    out = [first] if first else []
    n -= first or 0
    while n > 0:
        w = min(512, n)
        out.append(w)
        n -= w
    return out


def _dma_split_k(nc, out_tile, src, nsplit, hipri=None):
    """Split a [128, K, ...] transfer into nsplit k-range DMAs so it spreads
    across DMA engines (each engine moves ~22.5 B/ns; one big transfer is
    single-engine latency-bound)."""
    K = out_tile.shape[1]
    step = (K + nsplit - 1) // nsplit
    for k0 in range(0, K, step):
        k1 = min(k0 + step, K)
        if hipri is not None:
            with hipri.high_priority():
                nc.sync.dma_start(out=out_tile[:, k0:k1], in_=src[:, k0:k1])
        else:
            nc.sync.dma_start(out=out_tile[:, k0:k1], in_=src[:, k0:k1])


def _emit_mm3(nc, ps, wh_slab, wl_slab, x_tile, xc0, w, n_kt, first, last,
              use_wlo=True):
    """Accumulate sum_k w~[k].T @ x~[k] into ps[:, :w] via 3-product fp8
    DoubleRow instructions. wh/wl slabs: [128, n_kt(+pad), 128]; x_tile:
    [128, n_kt, 2, tokens] (hi/lo interleaved), token cols [xc0, xc0+w).
    n_kt may be odd: slabs carry a dup/zero pad k-tile (see module doc).
    use_wlo=False drops the w-residual product (2-product mode)."""
    xs = x_tile[:, :, :, xc0:xc0 + w]
    npair = n_kt // 2
    per = 3 if use_wlo else 2
    n3 = npair * per + (2 if n_kt % 2 else 0)
    i = 0
    for kp in range(npair):
        k = 2 * kp
        prods = [
            (wh_slab[:, k:k + 2, :], xs[:, k:k + 2, 0, :]),
            (wh_slab[:, k:k + 2, :], xs[:, k:k + 2, 1, :]),
        ]
        if use_wlo:
            prods.append((wl_slab[:, k:k + 2, :], xs[:, k:k + 2, 0, :]))
        for lhsT, rhs in prods:
            nc.tensor.matmul(ps[:, :w], lhsT, rhs,
                             start=(first and i == 0),
                             stop=(last and i == n3 - 1), perf_mode=DR)
            i += 1
    if n_kt % 2:
        k = n_kt - 1
        # (w_hi[k], w_hi[k]-dup) x (x_hi[k], x_lo[k])
        nc.tensor.matmul(ps[:, :w], wh_slab[:, k:k + 2, :], xs[:, k, :, :],
                         start=(first and i == 0), stop=False, perf_mode=DR)
        # (w_lo[k], zeros) x (x_hi[k], x_hi[k+1]-garbage*0)
        nc.tensor.matmul(ps[:, :w], wl_slab[:, k:k + 2, :],
                         xs[:, k:k + 2, 0, :],
                         start=False, stop=last, perf_mode=DR)


def _emit_s1(nc, pools, *, b, x_tile, act_tile, chunks, ogroup=None):
    """Stage 1: per gate-o-tile, compute ps_g/ps_u via 3-product DoubleRow,
    then evict: ACT Silu -> ag; DVE (ps_u/16)*ag -> stage; ACT cast -> act_hi;
    DVE stage-hi -> act_lo. act_tile: [128, act_kt, 2, ntok] fp8.

    ogroup: if set, loop chunk-outer within o-groups of that size so the PE
    rides the incoming x stream instead of stalling o-by-o (startup block)."""
    w1p, psp, stp = pools["w1"], pools["ps"], pools["stage"]
    tc = pools["tc"]
    n_go = b["n_go"]
    w1h_r = b["w1h"]
    w1l_r = b["w1l"]
    spans = []
    t0 = 0
    for w in chunks:
        spans.append((t0, w))
        t0 += w

    hi_t = b.get("hi_t")  # tokens < hi_t: full 3-product; rest: hi-only w

    def load_slabs(o, hipri):
        # slab-major DRAM layout: row ot*128+p holds that slab's (k, o) run
        # contiguously (2048B) — full DMA bus width, 1 descriptor per row
        slabs = []
        for ot in (o, n_go + o):
            for si, src in enumerate((w1h_r, w1l_r)):
                sl = w1p.tile([128, 16, 128], F8, tag="w1slab",
                              name=f"w1_{b['tag']}_{ot}_{si}")
                s = src[ot * 128:(ot + 1) * 128, :].rearrange(
                    "p (k o) -> p k o", k=16)
                if hipri:
                    with tc.high_priority():
                        nc.sync.dma_start(out=sl[:], in_=s)
                else:
                    nc.sync.dma_start(out=sl[:], in_=s)
                slabs.append(sl)
        return slabs

    def emit_o_chunk(o, slabs, ci, t0, w):
        gwh, gwl, uwh, uwl = slabs
        full = hi_t is None or t0 < hi_t
        ps_g = psp.tile([128, 512], F32, tag="ps", name=f"psg_{o}_{ci}")
        ps_u = psp.tile([128, 512], F32, tag="ps", name=f"psu_{o}_{ci}")
        _emit_mm3(nc, ps_g, gwh, gwl, x_tile, t0, w, 16, True, True,
                  use_wlo=full)
        _emit_mm3(nc, ps_u, uwh, uwl, x_tile, t0, w, 16, True, True,
                  use_wlo=full)
        ag = stp.tile([128, 512], F32, tag="stage", name=f"ag_{o}_{ci}")
        st = stp.tile([128, 512], F32, tag="stage", name=f"st_{o}_{ci}")
        nc.scalar.activation(out=ag[:, :w], in_=ps_g[:, :w], func=AF.Silu,
                             scale=S1_EVICT)
        nc.vector.scalar_tensor_tensor(
            out=st[:, :w], in0=ps_u[:, :w], scalar=S1_UP, in1=ag[:, :w],
            op0=ALU.mult, op1=ALU.mult)
        hi = act_tile[:, o, 0, t0:t0 + w]
        nc.scalar.activation(out=hi, in_=st[:, :w], func=AF.Copy)
        if full:  # act_lo only consumed by the full-precision s2 tiles
            nc.vector.tensor_sub(act_tile[:, o, 1, t0:t0 + w], st[:, :w], hi)

    if ogroup is None:
        for o in range(n_go):
            slabs = load_slabs(o, o <= 2 and b.get("hipri_slab"))
            for ci, (t0, w) in enumerate(spans):
                emit_o_chunk(o, slabs, ci, t0, w)
    else:
        for g0 in range(0, n_go, ogroup):
            os_ = range(g0, min(g0 + ogroup, n_go))
            slabs = {o: load_slabs(o, g0 == 0 and b.get("hipri_slab"))
                     for o in os_}
            for ci, (t0, w) in enumerate(spans):
                for o in os_:
                    emit_o_chunk(o, slabs[o], ci, t0, w)


def _prefetch_w2(nc, pools, b):
    """Load all of a block's w2 slabs (4 hc x hi/lo); emitted right after the
    block's s1 so they don't queue behind the next block's x transfers."""
    w2p = pools["w2"]
    w2h_r = b["w2h"].rearrange("(k p) h -> p k h", p=128)
    w2l_r = b["w2l"].rearrange("(k p) h -> p k h", p=128)
    res = b["w2_res"] if b.get("w2_res") is not None else b.setdefault(
        "w2_cache", [None] * 4)
    for hc in range(4):
        if res[hc] is not None:
            continue
        tag, bufs = ("w2res", 8) if b.get("w2_res") is not None else \
            ("w2slab", 8)
        sh = w2p.tile([128, b["act_kt"], 512], F8, tag=tag, bufs=bufs,
                      name=f"w2h_{b['tag']}_{hc}")
        sl = w2p.tile([128, b["act_kt"], 512], F8, tag=tag, bufs=bufs,
                      name=f"w2l_{b['tag']}_{hc}")
        nsp = max(1, b["act_kt"] // 4)
        _dma_split_k(nc, sh, w2h_r[:, :, hc * 512:(hc + 1) * 512], nsp)
        _dma_split_k(nc, sl, w2l_r[:, :, hc * 512:(hc + 1) * 512], nsp)
        res[hc] = (sh, sl)
    return res


def _emit_s2(nc, pools, *, b, act_tile, part):
    """Stage 2: out[t, hc] = sum_k act~[k].T @ w2~[k], 3-product DoubleRow
    with act (hi/lo) stationary and w2 slabs moving. Evict with per-token
    (expert) or constant (shared) scale to bf16, DMA out."""
    psp, outp = pools["ps2"], pools["out"]
    n_kt = b["act_kt"] - 1 if b["odd_kt"] else b["act_kt"]
    ntok = b["ntok"]
    hi_t = b.get("hi_t")
    slabs = _prefetch_w2(nc, pools, b)
    hc_list = {1: range(2), 2: range(2, 4)}[part]
    ntt = (ntok + 127) // 128
    for hc in hc_list:
        w2h_slab, w2l_slab = slabs[hc]
        for tt in range(ntt):
            r0 = tt * 128
            w = min(128, ntok - r0)
            full = hi_t is None or r0 < hi_t
            ps = psp.tile([128, 512], F32, tag="ps2", name=f"ps2_{hc}_{tt}")
            npair = n_kt // 2
            per = 3 if full else 1
            n3 = npair * per + (n_kt % 2) * (2 if full else 1)
            i = 0
            for kp in range(npair):
                k = 2 * kp
                prods = [
                    (act_tile[:, k:k + 2, 0, r0:r0 + w], w2h_slab[:, k:k + 2, :]),
                ]
                if full:
                    prods.append((act_tile[:, k:k + 2, 1, r0:r0 + w],
                                  w2h_slab[:, k:k + 2, :]))
                    prods.append((act_tile[:, k:k + 2, 0, r0:r0 + w],
                                  w2l_slab[:, k:k + 2, :]))
                for lhsT, rhs in prods:
                    nc.tensor.matmul(ps[:w, :], lhsT, rhs, start=(i == 0),
                                     stop=(i == n3 - 1), perf_mode=DR)
                    i += 1
            if n_kt % 2:
                k = n_kt - 1
                if full:
                    # (act_hi[k], act_lo[k]) x (w2h[k], w2h[k]-dup)
                    nc.tensor.matmul(ps[:w, :], act_tile[:, k, :, r0:r0 + w],
                                     w2h_slab[:, k:k + 2, :], start=False,
                                     stop=False, perf_mode=DR)
                    # (act_hi[k], pad-0) x (w2l[k], 0)
                    nc.tensor.matmul(ps[:w, :],
                                     act_tile[:, k:k + 2, 0, r0:r0 + w],
                                     w2l_slab[:, k:k + 2, :],
                                     start=False, stop=True, perf_mode=DR)
                else:
                    # (act_hi[k], pad-0) x (w2h[k], w2h[k]-dup): pad slot is 0
                    nc.tensor.matmul(ps[:w, :],
                                     act_tile[:, k:k + 2, 0, r0:r0 + w],
                                     w2h_slab[:, k:k + 2, :],
                                     start=False, stop=True, perf_mode=DR)
            ysb = outp.tile([128, 512], BF16, tag="ysb",
                            name=f"ysb_{b['tag']}_{hc}_{tt}")
            if b["cw"] is not None:
                nc.scalar.activation(
                    out=ysb[:w, :], in_=ps[:w, :], func=AF.Copy,
                    scale=b["cw"][:w, tt:tt + 1])
            else:
                nc.scalar.activation(out=ysb[:w, :], in_=ps[:w, :],
                                     func=AF.Copy, scale=S2_EVICT)
            nc.sync.dma_start(
                out=b["out"][b["row0"] + r0: b["row0"] + r0 + w,
                             hc * 512:(hc + 1) * 512],
                in_=ysb[:w, :])


def _build(cap, order=(0, 1, 2, 3), first_chunk=256, defer_parts=(2,),
           ogroup0=None, w1bufs=8, hi_tiles=1):
    nc = bacc.Bacc("TRN2", target_bir_lowering=False, debug=False)

    cwcols = (cap + 127) // 128
    aps = {}
    for j in range(2):
        aps[f"xs{j}"] = nc.dram_tensor(f"xs{j}", [H, 2, cap], F8,
                                       kind="ExternalInput").ap()
        aps[f"w1h{j}"] = nc.dram_tensor(f"w1h{j}", [I2, H], F8,
                                        kind="ExternalInput").ap()
        aps[f"w1l{j}"] = nc.dram_tensor(f"w1l{j}", [I2, H], F8,
                                        kind="ExternalInput").ap()
        aps[f"w2h{j}"] = nc.dram_tensor(f"w2h{j}", [1536, H], F8,
                                        kind="ExternalInput").ap()
        aps[f"w2l{j}"] = nc.dram_tensor(f"w2l{j}", [1536, H], F8,
                                        kind="ExternalInput").ap()
        aps[f"cw{j}"] = nc.dram_tensor(f"cw{j}", [cwcols * 128], F32,
                                       kind="ExternalInput").ap()
        aps[f"y{j}"] = nc.dram_tensor(f"y{j}", [cap, H], BF16,
                                      kind="ExternalOutput").ap()
    aps["xt"] = nc.dram_tensor("xt", [H, 2, T], F8, kind="ExternalInput").ap()
    aps["sw1h"] = nc.dram_tensor("sw1h", [2 * SSLP, H], F8,
                                 kind="ExternalInput").ap()
    aps["sw1l"] = nc.dram_tensor("sw1l", [2 * SSLP, H], F8,
                                 kind="ExternalInput").ap()
    aps["sw2h"] = nc.dram_tensor("sw2h", [512, H], F8,
                                 kind="ExternalInput").ap()
    aps["sw2l"] = nc.dram_tensor("sw2l", [512, H], F8,
                                 kind="ExternalInput").ap()
    aps["ys"] = nc.dram_tensor("ys", [T, H], BF16, kind="ExternalOutput").ap()

    import contextlib
    with tile.TileContext(nc) as tc, contextlib.ExitStack() as ctx:
        pools = {
            "x": ctx.enter_context(tc.tile_pool(name="x", bufs=2)),
            "w1": ctx.enter_context(tc.tile_pool(name="w1", bufs=w1bufs)),
            "w2": ctx.enter_context(tc.tile_pool(name="w2", bufs=4)),
            "act": ctx.enter_context(tc.tile_pool(name="act", bufs=2)),
            "stage": ctx.enter_context(tc.tile_pool(name="stage", bufs=4)),
            "out": ctx.enter_context(tc.tile_pool(name="out", bufs=6)),
            # separate s1/s2 PSUM pools: the cross-block s2 deferral must
            # never be starved of PSUM slots by the next block's stalled s1
            "ps": ctx.enter_context(tc.tile_pool(name="ps", bufs=4,
                                                 space="PSUM")),
            "ps2": ctx.enter_context(tc.tile_pool(name="ps2", bufs=4,
                                                  space="PSUM")),
            "misc": ctx.enter_context(tc.tile_pool(name="misc", bufs=2)),
        }
        pools["tc"] = tc
        cw_tiles = {}

        def get_cw(j):
            if j not in cw_tiles:
                cw_r = aps[f"cw{j}"].rearrange("(n p) -> p n", p=128)
                cw_tiles[j] = pools["misc"].tile([128, cwcols], F32,
                                                 tag=f"cw{j}", name=f"cw{j}_t")
                with tc.high_priority():
                    nc.sync.dma_start(out=cw_tiles[j][:], in_=cw_r[:])
            return cw_tiles[j]

        get_cw(0)  # tiny; load up front so s2 evictions never wait on them
        get_cw(1)

        shared_res = [None, None, None, None]
        all_blocks = []
        for j in range(2):
            all_blocks.append(dict(
                tag=f"e{j}", n_go=11, act_kt=12, odd_kt=True, ntok=cap,
                x_ap=aps[f"xs{j}"], x_off=0,
                w1h=aps[f"w1h{j}"], w1l=aps[f"w1l{j}"],
                w2h=aps[f"w2h{j}"], w2l=aps[f"w2l{j}"],
                out=aps[f"y{j}"], row0=0, cw_j=j, w2_res=None,
                hipri_slab=True,
                hi_t=None if hi_tiles is None else 128 * hi_tiles,
            ))
        for half in range(2):
            all_blocks.append(dict(
                tag=f"sh{half}", n_go=3, act_kt=4, odd_kt=True, ntok=1024,
                x_ap=aps["xt"], x_off=half * 1024,
                w1h=aps["sw1h"], w1l=aps["sw1l"],
                w2h=aps["sw2h"], w2l=aps["sw2l"],
                out=aps["ys"], row0=half * 1024, cw_j=None,
                w2_res=shared_res, hipri_slab=True,
            ))

        blocks = [all_blocks[i] for i in order]

        def s1_chunks(n):
            b = blocks[n]
            hi_t = b.get("hi_t")
            if hi_t:  # chunk boundary must align with the precision boundary
                return [hi_t] + _nchunks(b["ntok"] - hi_t)
            return _nchunks(b["ntok"], first=first_chunk if n == 0 else None)

        def load_x(n, hipri_first_only=False):
            b = blocks[n]
            x_r = b["x_ap"].rearrange("(k p) s t -> p k s t", p=128)
            xt_tile = pools["x"].tile([128, 16, 2, b["ntok"]], F8, tag="xsel",
                                      name=f"x_{b['tag']}")
            t0 = 0
            for ci, w in enumerate(s1_chunks(n)):
                for s in range(2):  # per-slot DMA keeps APs 3-D (balancer cap)
                    src = x_r[:, :, s, b["x_off"] + t0: b["x_off"] + t0 + w]
                    dst = xt_tile[:, :, s, t0:t0 + w]
                    hp = tc if (ci == 0 or not hipri_first_only) else None
                    _dma_split_k(nc, dst, src, 2, hipri=hp)
                t0 += w
            return xt_tile

        def emit_s2_part(b, act_tile, part):
            if "cw" not in b:
                b["cw"] = None if b["cw_j"] is None else get_cw(b["cw_j"])
            _emit_s2(nc, pools, b=b, act_tile=act_tile, part=part)

        # Emit s1(n), then block n+1's x-load, then the previous block's
        # deferred s2 half, then s2(n) part 1: the next x-load overlaps s2(n)
        # compute instead of queueing behind its weight slabs.
        x_tiles = [load_x(0, hipri_first_only=True)]
        deferred = None
        for n, b in enumerate(blocks):
            act_tile = pools["act"].tile([128, b["act_kt"], 2, b["ntok"]], F8,
                                         tag="act", name=f"act_{b['tag']}")
            # zero the pad k-tile (hi slot is read by the odd-k leftover
            # instruction; lo slot never read)
            nc.gpsimd.memset(act_tile[:, b["act_kt"] - 1, 0, :], 0.0)
            _emit_s1(nc, pools, b=b, x_tile=x_tiles[n], act_tile=act_tile,
                     chunks=s1_chunks(n), ogroup=ogroup0 if n == 0 else None)
            _prefetch_w2(nc, pools, b)
            if n + 1 < len(blocks):
                x_tiles.append(load_x(n + 1))
            if deferred is not None:
                for p in defer_parts:
                    emit_s2_part(*deferred, part=p)
                deferred = None
            for p in (1, 2):
                if p not in defer_parts:
                    emit_s2_part(b, act_tile, part=p)
            deferred = (b, act_tile)
        if deferred is not None:
            for p in defer_parts:
                emit_s2_part(*deferred, part=p)

    nc.compile()
    return nc


def _route(xf, gate_w):
    """Host router: fp32 softmax + top-6, matching jax bitwise when possible."""
    try:
        import jax
        import jax.numpy as jnp

        cpu = jax.devices("cpu")[0]
        with jax.default_device(cpu):
            logits = jnp.asarray(xf) @ jnp.asarray(gate_w).T
            probs = jax.nn.softmax(logits.astype(jnp.float32), axis=-1)
            _, sel = jax.lax.top_k(probs, TOPK)
        return np.asarray(probs), np.asarray(sel)
    except Exception:
        logits = xf @ gate_w.T
        m = logits.max(axis=-1, keepdims=True)
        e = np.exp(logits - m, dtype=np.float32)
        probs = e / e.sum(axis=-1, keepdims=True)
        sel = np.argsort(-probs, axis=-1, kind="stable")[:, :TOPK]
        return probs, sel


def _split8(a, s):
    """-> (hi, lo) fp8 arrays with a*s ~= hi + lo."""
    sa = (a * s).astype(np.float32)
    hi = sa.astype(F8NP)
    lo = (sa - hi.astype(np.float32)).astype(F8NP)
    return hi, lo


def _slab_major(w):
    """[H, O] -> [O, H] slab-major: row ot*128+p holds slab ot's (k, o) run
    contiguously, so each w1 slab DMA moves 2048B-contiguous rows."""
    Hd, O = w.shape
    return np.ascontiguousarray(
        w.reshape(Hd // 128, 128, O // 128, 128).transpose(2, 1, 0, 3)
        .reshape(O, Hd))


def kernel(x, gate_w, w1, w2, shared_w1, shared_w2):
    x = np.asarray(x, np.float32)
    gate_w = np.asarray(gate_w, np.float32)
    w1 = np.asarray(w1, np.float32)
    w2 = np.asarray(w2, np.float32)
    shared_w1 = np.asarray(shared_w1, np.float32)
    shared_w2 = np.asarray(shared_w2, np.float32)

    B, S, Hd = x.shape
    xf = np.ascontiguousarray(x.reshape(-1, Hd))  # [T, H]

    probs, sel = _route(xf, gate_w)
    onehot = np.zeros((T, E), bool)
    onehot[np.arange(T)[:, None], sel] = True
    # sort each expert's tokens by routing weight (descending) so the first
    # token tiles hold the high-weight tokens that get full 3-product
    # precision; low-weight tokens use the cheap hi-only products
    idx_e = []
    for e in range(E):
        ix = np.nonzero(onehot[:, e])[0]
        idx_e.append(ix[np.argsort(-probs[ix, e], kind="stable")])
    counts = np.array([len(ix) for ix in idx_e])

    cap = CAP0
    while counts.max() > cap:
        cap += 64
    if cap not in _compiled:
        _compiled[cap] = _build(cap, order=(0, 2, 3, 1), defer_parts=(),
                                hi_tiles=1)
    nc = _compiled[cap]

    # quantize x once: [T, H] hi/lo
    xq_hi, xq_lo = _split8(xf, SX)
    xt2 = np.empty((H, 2, T), F8NP)
    xt2[:, 0, :] = xq_hi.T
    xt2[:, 1, :] = xq_lo.T

    cwcols = (cap + 127) // 128
    in_maps = []
    for c in range(NCORES):
        m = {"xt": xt2}
        for j in range(2):
            e = 2 * c + j
            ix = idx_e[e]
            xs2 = np.zeros((H, 2, cap), F8NP)
            xs2[:, 0, :len(ix)] = xq_hi[ix].T
            xs2[:, 1, :len(ix)] = xq_lo[ix].T
            m[f"xs{j}"] = xs2
            hi, lo = _split8(w1[e].T, SW1)  # [H, I2]
            m[f"w1h{j}"] = _slab_major(hi)
            m[f"w1l{j}"] = _slab_major(lo)
            hi, lo = _split8(w2[e].T, SW2)  # [I, H]
            w2h = np.zeros((1536, H), F8NP)
            w2l = np.zeros((1536, H), F8NP)
            w2h[:I] = hi
            w2l[:I] = lo
            w2h[I:] = hi[-128:]  # dup of k-tile 10 for the odd-k leftover
            m[f"w2h{j}"] = w2h
            m[f"w2l{j}"] = w2l
            cw = np.zeros(cwcols * 128, np.float32)
            cw[: len(ix)] = probs[ix, e] * S2_EVICT
            m[f"cw{j}"] = cw
        sl = slice(SSL * c, SSL * (c + 1))
        sg = np.zeros((H, SSLP), np.float32)
        su = np.zeros((H, SSLP), np.float32)
        sg[:, :SSL] = shared_w1[sl].T
        su[:, :SSL] = shared_w1[ISH + SSL * c: ISH + SSL * (c + 1)].T
        hi_g, lo_g = _split8(sg, SW1)
        hi_u, lo_u = _split8(su, SW1)
        m["sw1h"] = _slab_major(np.concatenate([hi_g, hi_u], axis=1))
        m["sw1l"] = _slab_major(np.concatenate([lo_g, lo_u], axis=1))
        s2w = np.zeros((512, H), np.float32)
        s2w[:SSL] = shared_w2[:, sl].T
        hi, lo = _split8(s2w, SW2)
        hi[SSLP:] = hi[SSLP - 128: SSLP]  # dup k-tile 2
        lo[SSLP:] = 0
        m["sw2h"] = hi
        m["sw2l"] = lo
        in_maps.append(m)

    try:
        res = run_bass_kernel_spmd(nc, in_maps, list(range(NCORES)))
    except ModuleNotFoundError:
        os.environ["BASS_NEVER_TRACE"] = "1"
        res = run_bass_kernel_spmd(nc, in_maps, list(range(NCORES)))
    global last_result
    last_result = res

    out = np.zeros((T, H), np.float32)
    for c in range(NCORES):
        out += res.results[c]["ys"].astype(np.float32)
        for j in range(2):
            e = 2 * c + j
            ix = idx_e[e]
            out[ix] += res.results[c][f"y{j}"][: len(ix)].astype(np.float32)

    return out.reshape(B, S, Hd)


# revision 67
# speedup vs baseline: 1.8882x; 1.0345x over previous
"""DeepseekMoE layer on 8 TRN2 NeuronCores — expert-parallel Bass/Tile kernel.

Strategy (self-contained, shapes hardcoded for this problem):
  H=2048, T=2048 tokens, E=16 experts, top-6, I=1408, shared IS=2816.

  Sharding (done on host inside kernel(), per the full-input contract):
    - Router (softmax + top-6) computed on host in fp32 (jax-on-CPU when
      available so near-tie selections match the jax reference bitwise)
      -> per-expert token lists (the "all-to-all dispatch" decision).
    - Core c owns experts 2c, 2c+1 (capacity-padded to CAP tokens each);
      shared expert sharded over its intermediate dim (352 rows per core,
      padded to 384 = 3*128).
    - Each core returns per-expert outputs [CAP, H] bf16 (pre-scaled by
      routing weights) and a dense shared partial [T, H] bf16; the host
      scatter-adds in fp32.

  Arithmetic: all matmuls run as fp8(e4m3) DoubleRow pairs at 0.5 cyc/row,
  using a hi+lo residual decomposition of every operand:
      a*s ~= a_hi + a_lo   (a_hi = fp8(a*s), a_lo = fp8(a*s - a_hi))
      a*b*s_a*s_b ~= a_hi*b_hi + a_lo*b_hi + a_hi*b_lo     (lo*lo dropped)
  Three DoubleRow instructions per k-tile pair = 0.75 cyc/row/k-tile, a
  1.33x speedup over bf16/fp32r with ~bf16 effective precision (measured
  end-to-end rel err 2.6e-3 vs the 2e-2 gate).

  Scales: x*4, w1*64, w2*128. Stage-1 PSUM: gate = g*256 (Silu evicted with
  scale 2^-8); up = u*256, fused DVE (ps_u * 1/16) * silu_g = act*16, which
  is split hi/lo to fp8 for stage 2. Stage-2 PSUM = y*2048; eviction scale
  folds 2^-11 into the per-token routing weight (or a constant for the
  shared expert). Odd k-tile counts (11 expert, 3 shared) are handled by
  host-side slab rows: hi-slab gets a duplicated last k-tile, lo-slab gets
  zeros, plus a one-time memset of the act tile's pad k-tile.
"""

import os
import sys

sys.path.insert(0, "/opt/trn_rl_repo")

import numpy as np
import ml_dtypes

import concourse.bass as bass  # noqa: F401
import concourse.tile as tile
from concourse import bacc, mybir
from concourse.bass_utils import run_bass_kernel_spmd

H = 2048
T = 2048
E = 16
TOPK = 6
I2 = 2816  # 2*I
I = 1408
ISH = 2816  # shared intermediate (per gate/up half)
NCORES = 8
CAP0 = 832  # per-expert token capacity; grown in 64s if exceeded
SSL = 352  # shared-intermediate slice per core
SSLP = 384  # padded to 3*128

SX, SW1, SW2, SACT = 4.0, 64.0, 128.0, 16.0
S1_EVICT = 1.0 / (SW1 * SX)  # 2^-8
S1_UP = SACT / (SW1 * SX)  # 1/16
S2_EVICT = 1.0 / (SACT * SW2)  # 2^-11

F8NP = ml_dtypes.float8_e4m3
BF16NP = ml_dtypes.bfloat16
F32 = mybir.dt.float32
F8 = mybir.dt.float8e4
BF16 = mybir.dt.bfloat16
AF = mybir.ActivationFunctionType
ALU = mybir.AluOpType
DR = mybir.MatmulPerfMode.DoubleRow

_compiled = {}
last_result = None  # BassKernelResults of the most recent run (for profiling)


def _nchunks(n, first=None):
    """Split n into <=512 free-dim chunks; optional smaller first chunk so the
    first PSUM group starts after a fraction of the x block has landed."""
    out = [first] if first else []
    n -= first or 0
    while n > 0:
        w = min(512, n)
        out.append(w)
        n -= w
    return out


def _dma_split_k(nc, out_tile, src, nsplit, hipri=None):
    """Split a [128, K, ...] transfer into nsplit k-range DMAs so it spreads
    across DMA engines (each engine moves ~22.5 B/ns; one big transfer is
    single-engine latency-bound)."""
    K = out_tile.shape[1]
    step = (K + nsplit - 1) // nsplit
    for k0 in range(0, K, step):
        k1 = min(k0 + step, K)
        if hipri is not None:
            with hipri.high_priority():
                nc.sync.dma_start(out=out_tile[:, k0:k1], in_=src[:, k0:k1])
        else:
            nc.sync.dma_start(out=out_tile[:, k0:k1], in_=src[:, k0:k1])


def _emit_mm3(nc, ps, wh_slab, wl_slab, x_tile, xc0, w, n_kt, first, last,
              use_wlo=True):
    """Accumulate sum_k w~[k].T @ x~[k] into ps[:, :w] via 3-product fp8
    DoubleRow instructions. wh/wl slabs: [128, n_kt(+pad), 128]; x_tile:
    [128, n_kt, 2, tokens] (hi/lo interleaved), token cols [xc0, xc0+w).
    n_kt may be odd: slabs carry a dup/zero pad k-tile (see module doc).
    use_wlo=False drops the w-residual product (2-product mode)."""
    xs = x_tile[:, :, xc0:xc0 + w, :]  # [128, kt, w, 2(hi/lo)]
    npair = n_kt // 2
    per = 3 if use_wlo else 2
    n3 = npair * per + (2 if n_kt % 2 else 0)
    i = 0
    for kp in range(npair):
        k = 2 * kp
        prods = [
            (wh_slab[:, k:k + 2, :], xs[:, k:k + 2, :, 0]),
            (wh_slab[:, k:k + 2, :], xs[:, k:k + 2, :, 1]),
        ]
        if use_wlo:
            prods.append((wl_slab[:, k:k + 2, :], xs[:, k:k + 2, :, 0]))
        for lhsT, rhs in prods:
            nc.tensor.matmul(ps[:, :w], lhsT, rhs,
                             start=(first and i == 0),
                             stop=(last and i == n3 - 1), perf_mode=DR)
            i += 1
    if n_kt % 2:
        k = n_kt - 1
        # (w_hi[k], w_hi[k]-dup) x (x_hi[k], x_lo[k]): slot dim from hi/lo
        nc.tensor.matmul(ps[:, :w], wh_slab[:, k:k + 2, :],
                         xs[:, k, :, :].rearrange("p w s -> p s w"),
                         start=(first and i == 0), stop=False, perf_mode=DR)
        # (w_lo[k], zeros) x (x_hi[k], x_hi[k+1]-garbage*0)
        nc.tensor.matmul(ps[:, :w], wl_slab[:, k:k + 2, :],
                         xs[:, k:k + 2, :, 0],
                         start=False, stop=last, perf_mode=DR)


def _emit_s1(nc, pools, *, b, x_tile, act_tile, chunks, ogroup=None):
    """Stage 1: per gate-o-tile, compute ps_g/ps_u via 3-product DoubleRow,
    then evict: ACT Silu -> ag; DVE (ps_u/16)*ag -> stage; ACT cast -> act_hi;
    DVE stage-hi -> act_lo. act_tile: [128, act_kt, 2, ntok] fp8.

    ogroup: if set, loop chunk-outer within o-groups of that size so the PE
    rides the incoming x stream instead of stalling o-by-o (startup block)."""
    w1p, psp, stp = pools["w1"], pools["ps"], pools["stage"]
    tc = pools["tc"]
    n_go = b["n_go"]
    spans = []
    t0 = 0
    for w in chunks:
        spans.append((t0, w))
        t0 += w

    hi_t = b.get("hi_t")  # tokens < hi_t: full 3-product; rest: hi-only w

    def load_slabs(o, hipri):
        # w1a rows o*512..o*512+512 hold the o-tile's 4 slabs (gwh, gwl,
        # uwh, uwl), each slab-major with its (k, o) run contiguous (2048B):
        # one DMA loads everything the o-tile needs
        t = w1p.tile([128, 4, 16, 128], F8, tag="w1slab",
                     name=f"w1_{b['tag']}_{o}")
        src = b["w1a"][o * 512:(o + 1) * 512, :].rearrange(
            "(s p) c -> p s c", p=128)
        if hipri:
            with tc.high_priority():
                nc.sync.dma_start(out=t[:], in_=src)
        else:
            nc.sync.dma_start(out=t[:], in_=src)
        return [t[:, 0], t[:, 1], t[:, 2], t[:, 3]]

    def emit_o_chunk(o, slabs, ci, t0, w):
        gwh, gwl, uwh, uwl = slabs
        full = hi_t is None or t0 < hi_t
        ps_g = psp.tile([128, 512], F32, tag="ps", name=f"psg_{o}_{ci}")
        ps_u = psp.tile([128, 512], F32, tag="ps", name=f"psu_{o}_{ci}")
        _emit_mm3(nc, ps_g, gwh, gwl, x_tile, t0, w, 16, True, True,
                  use_wlo=full)
        _emit_mm3(nc, ps_u, uwh, uwl, x_tile, t0, w, 16, True, True,
                  use_wlo=full)
        ag = stp.tile([128, 512], F32, tag="stage", name=f"ag_{o}_{ci}")
        st = stp.tile([128, 512], F32, tag="stage", name=f"st_{o}_{ci}")
        nc.scalar.activation(out=ag[:, :w], in_=ps_g[:, :w], func=AF.Silu,
                             scale=S1_EVICT)
        nc.vector.scalar_tensor_tensor(
            out=st[:, :w], in0=ps_u[:, :w], scalar=S1_UP, in1=ag[:, :w],
            op0=ALU.mult, op1=ALU.mult)
        hi = act_tile[:, o, 0, t0:t0 + w]
        nc.scalar.activation(out=hi, in_=st[:, :w], func=AF.Copy)
        if full:  # act_lo only consumed by the full-precision s2 tiles
            nc.vector.tensor_sub(act_tile[:, o, 1, t0:t0 + w], st[:, :w], hi)

    if ogroup is None:
        for o in range(n_go):
            slabs = load_slabs(o, o <= 2 and b.get("hipri_slab"))
            for ci, (t0, w) in enumerate(spans):
                emit_o_chunk(o, slabs, ci, t0, w)
    else:
        for g0 in range(0, n_go, ogroup):
            os_ = range(g0, min(g0 + ogroup, n_go))
            slabs = {o: load_slabs(o, g0 == 0 and b.get("hipri_slab"))
                     for o in os_}
            for ci, (t0, w) in enumerate(spans):
                for o in os_:
                    emit_o_chunk(o, slabs[o], ci, t0, w)


def _prefetch_w2(nc, pools, b):
    """Load all of a block's w2 slabs (4 hc, hi+lo packed in one DMA each);
    emitted right after the block's s1 so they don't queue behind the next
    block's x transfers."""
    w2p = pools["w2"]
    kt = b["act_kt"]
    w2a_r = b["w2a"].rearrange("(s k p) h -> p s k h", s=2, p=128)
    res = b["w2_res"] if b.get("w2_res") is not None else b.setdefault(
        "w2_cache", [None] * 4)
    for hc in range(4):
        if res[hc] is not None:
            continue
        tag, bufs = ("w2res", 8) if b.get("w2_res") is not None else \
            ("w2slab", 6)
        sh = w2p.tile([128, kt, 512], F8, tag=tag, bufs=bufs,
                      name=f"w2h_{b['tag']}_{hc}")
        sl = w2p.tile([128, kt, 512], F8, tag=tag, bufs=bufs,
                      name=f"w2l_{b['tag']}_{hc}")
        nc.sync.dma_start(out=sh[:],
                          in_=w2a_r[:, 0, :, hc * 512:(hc + 1) * 512])
        nc.sync.dma_start(out=sl[:],
                          in_=w2a_r[:, 1, :, hc * 512:(hc + 1) * 512])
        res[hc] = (sh, sl)
    return res


def _emit_s2(nc, pools, *, b, act_tile, part):
    """Stage 2: out[t, hc] = sum_k act~[k].T @ w2~[k], 3-product DoubleRow
    with act (hi/lo) stationary and w2 slabs moving. Evict with per-token
    (expert) or constant (shared) scale to bf16, DMA out."""
    psp, outp = pools["ps2"], pools["out"]
    n_kt = b["act_kt"] - 1 if b["odd_kt"] else b["act_kt"]
    ntok = b["ntok"]
    hi_t = b.get("hi_t")
    slabs = _prefetch_w2(nc, pools, b)
    ntt = (ntok + 127) // 128
    nt1 = (ntt + 1) // 2
    tt_list = {1: range(nt1), 2: range(nt1, ntt)}[part]
    for tt in tt_list:
        r0 = tt * 128
        w = min(128, ntok - r0)
        full = hi_t is None or r0 < hi_t
        ysb = outp.tile([128, 2048], BF16, tag="ysb",
                        name=f"ysb_{b['tag']}_{tt}")
        for hc in range(4):
            w2h_slab, w2l_slab = slabs[hc]
            ps = psp.tile([128, 512], F32, tag="ps2", name=f"ps2_{hc}_{tt}")
            npair = n_kt // 2
            per = 3 if full else 1
            n3 = npair * per + (n_kt % 2) * (2 if full else 1)
            i = 0
            for kp in range(npair):
                k = 2 * kp
                prods = [
                    (act_tile[:, k:k + 2, 0, r0:r0 + w], w2h_slab[:, k:k + 2, :]),
                ]
                if full:
                    prods.append((act_tile[:, k:k + 2, 1, r0:r0 + w],
                                  w2h_slab[:, k:k + 2, :]))
                    prods.append((act_tile[:, k:k + 2, 0, r0:r0 + w],
                                  w2l_slab[:, k:k + 2, :]))
                for lhsT, rhs in prods:
                    nc.tensor.matmul(ps[:w, :], lhsT, rhs, start=(i == 0),
                                     stop=(i == n3 - 1), perf_mode=DR)
                    i += 1
            if n_kt % 2:
                k = n_kt - 1
                if full:
                    # (act_hi[k], act_lo[k]) x (w2h[k], w2h[k]-dup)
                    nc.tensor.matmul(ps[:w, :], act_tile[:, k, :, r0:r0 + w],
                                     w2h_slab[:, k:k + 2, :], start=False,
                                     stop=False, perf_mode=DR)
                    # (act_hi[k], pad-0) x (w2l[k], 0)
                    nc.tensor.matmul(ps[:w, :],
                                     act_tile[:, k:k + 2, 0, r0:r0 + w],
                                     w2l_slab[:, k:k + 2, :],
                                     start=False, stop=True, perf_mode=DR)
                else:
                    # (act_hi[k], pad-0) x (w2h[k], w2h[k]-dup): pad slot is 0
                    nc.tensor.matmul(ps[:w, :],
                                     act_tile[:, k:k + 2, 0, r0:r0 + w],
                                     w2h_slab[:, k:k + 2, :],
                                     start=False, stop=True, perf_mode=DR)
            # alternate evict engine so PSUM drain never paces the PE
            dst = ysb[:w, hc * 512:(hc + 1) * 512]
            if b["cw"] is not None:
                if hc % 2 == 0:
                    nc.scalar.activation(out=dst, in_=ps[:w, :], func=AF.Copy,
                                         scale=b["cw"][:w, tt:tt + 1])
                else:
                    nc.vector.tensor_scalar_mul(dst, ps[:w, :],
                                                b["cw"][:w, tt:tt + 1])
            else:
                if hc % 2 == 0:
                    nc.scalar.activation(out=dst, in_=ps[:w, :], func=AF.Copy,
                                         scale=S2_EVICT)
                else:
                    nc.vector.tensor_scalar_mul(dst, ps[:w, :], S2_EVICT)
        nc.sync.dma_start(
            out=b["out"][b["row0"] + r0: b["row0"] + r0 + w, :],
            in_=ysb[:w, :])


def _build(cap, order=(0, 1, 2, 3), first_chunk=256, defer_parts=(2,),
           ogroup0=None, w1bufs=3, hi_tiles=1):
    nc = bacc.Bacc("TRN2", target_bir_lowering=False, debug=False)

    cwcols = (cap + 127) // 128
    aps = {}
    for j in range(2):
        aps[f"xs{j}"] = nc.dram_tensor(f"xs{j}", [H, cap, 2], F8,
                                       kind="ExternalInput").ap()
        aps[f"w1a{j}"] = nc.dram_tensor(f"w1a{j}", [2 * I2, H], F8,
                                        kind="ExternalInput").ap()
        aps[f"w2a{j}"] = nc.dram_tensor(f"w2a{j}", [3072, H], F8,
                                        kind="ExternalInput").ap()
        aps[f"cw{j}"] = nc.dram_tensor(f"cw{j}", [cwcols * 128], F32,
                                       kind="ExternalInput").ap()
        aps[f"y{j}"] = nc.dram_tensor(f"y{j}", [cap, H], BF16,
                                      kind="ExternalOutput").ap()
    aps["xt"] = nc.dram_tensor("xt", [H, T, 2], F8, kind="ExternalInput").ap()
    aps["sw1a"] = nc.dram_tensor("sw1a", [4 * SSLP, H], F8,
                                 kind="ExternalInput").ap()
    aps["sw2a"] = nc.dram_tensor("sw2a", [1024, H], F8,
                                 kind="ExternalInput").ap()
    aps["ys"] = nc.dram_tensor("ys", [T, H], BF16, kind="ExternalOutput").ap()

    import contextlib
    with tile.TileContext(nc) as tc, contextlib.ExitStack() as ctx:
        pools = {
            "x": ctx.enter_context(tc.tile_pool(name="x", bufs=2)),
            "w1": ctx.enter_context(tc.tile_pool(name="w1", bufs=w1bufs)),
            "w2": ctx.enter_context(tc.tile_pool(name="w2", bufs=4)),
            "act": ctx.enter_context(tc.tile_pool(name="act", bufs=2)),
            "stage": ctx.enter_context(tc.tile_pool(name="stage", bufs=4)),
            "out": ctx.enter_context(tc.tile_pool(name="out", bufs=3)),
            # separate s1/s2 PSUM pools: the cross-block s2 deferral must
            # never be starved of PSUM slots by the next block's stalled s1
            "ps": ctx.enter_context(tc.tile_pool(name="ps", bufs=4,
                                                 space="PSUM")),
            "ps2": ctx.enter_context(tc.tile_pool(name="ps2", bufs=4,
                                                  space="PSUM")),
            "misc": ctx.enter_context(tc.tile_pool(name="misc", bufs=2)),
        }
        pools["tc"] = tc
        cw_tiles = {}

        def get_cw(j):
            if j not in cw_tiles:
                cw_r = aps[f"cw{j}"].rearrange("(n p) -> p n", p=128)
                cw_tiles[j] = pools["misc"].tile([128, cwcols], F32,
                                                 tag=f"cw{j}", name=f"cw{j}_t")
                with tc.high_priority():
                    nc.sync.dma_start(out=cw_tiles[j][:], in_=cw_r[:])
            return cw_tiles[j]

        get_cw(0)  # tiny; load up front so s2 evictions never wait on them
        get_cw(1)

        shared_res = [None, None, None, None]
        all_blocks = []
        for j in range(2):
            all_blocks.append(dict(
                tag=f"e{j}", n_go=11, act_kt=12, odd_kt=True, ntok=cap,
                x_ap=aps[f"xs{j}"], x_off=0,
                w1a=aps[f"w1a{j}"], w2a=aps[f"w2a{j}"],
                out=aps[f"y{j}"], row0=0, cw_j=j, w2_res=None,
                hipri_slab=True,
                hi_t=None if hi_tiles is None else 128 * hi_tiles,
            ))
        for half in range(2):
            all_blocks.append(dict(
                tag=f"sh{half}", n_go=3, act_kt=4, odd_kt=True, ntok=1024,
                x_ap=aps["xt"], x_off=half * 1024,
                w1a=aps["sw1a"], w2a=aps["sw2a"],
                out=aps["ys"], row0=half * 1024, cw_j=None,
                w2_res=shared_res, hipri_slab=True,
            ))

        blocks = [all_blocks[i] for i in order]

        def s1_chunks(n):
            b = blocks[n]
            hi_t = b.get("hi_t")
            if hi_t:  # chunk boundary must align with the precision boundary
                return [hi_t] + _nchunks(b["ntok"] - hi_t)
            return _nchunks(b["ntok"], first=first_chunk if n == 0 else None)

        def load_x(n, first_hipri=False):
            # only block 0's first chunk is urgent; later blocks' x loads are
            # prefetches that must NOT outrank the current block's stream
            b = blocks[n]
            x_r = b["x_ap"].rearrange("(k p) t s -> p k t s", p=128)
            xt_tile = pools["x"].tile([128, 16, b["ntok"], 2], F8, tag="xsel",
                                      name=f"x_{b['tag']}")
            t0 = 0
            for ci, w in enumerate(s1_chunks(n)):
                src = x_r[:, :, b["x_off"] + t0: b["x_off"] + t0 + w, :]
                dst = xt_tile[:, :, t0:t0 + w, :]
                if ci == 0 and first_hipri:
                    with tc.high_priority():
                        nc.sync.dma_start(out=dst, in_=src)
                else:
                    nc.sync.dma_start(out=dst, in_=src)
                t0 += w
            return xt_tile

        def emit_s2_part(b, act_tile, part):
            if "cw" not in b:
                b["cw"] = None if b["cw_j"] is None else get_cw(b["cw_j"])
            _emit_s2(nc, pools, b=b, act_tile=act_tile, part=part)

        # Emit s1(n), then block n+1's x-load, then the previous block's
        # deferred s2 half, then s2(n) part 1: the next x-load overlaps s2(n)
        # compute instead of queueing behind its weight slabs.
        x_tiles = [load_x(0, first_hipri=True)]
        deferred = None
        for n, b in enumerate(blocks):
            act_tile = pools["act"].tile([128, b["act_kt"], 2, b["ntok"]], F8,
                                         tag="act", name=f"act_{b['tag']}")
            # zero the pad k-tile (hi slot is read by the odd-k leftover
            # instruction; lo slot never read)
            nc.gpsimd.memset(act_tile[:, b["act_kt"] - 1, 0, :], 0.0)
            _emit_s1(nc, pools, b=b, x_tile=x_tiles[n], act_tile=act_tile,
                     chunks=s1_chunks(n), ogroup=ogroup0 if n == 0 else None)
            _prefetch_w2(nc, pools, b)
            if n + 1 < len(blocks):
                x_tiles.append(load_x(n + 1))
            if deferred is not None:
                for p in defer_parts:
                    emit_s2_part(*deferred, part=p)
                deferred = None
            for p in (1, 2):
                if p not in defer_parts:
                    emit_s2_part(b, act_tile, part=p)
            deferred = (b, act_tile)
        if deferred is not None:
            for p in defer_parts:
                emit_s2_part(*deferred, part=p)

    nc.compile()
    return nc


def _route(xf, gate_w):
    """Host router: fp32 softmax + top-6, matching jax bitwise when possible."""
    try:
        import jax
        import jax.numpy as jnp

        cpu = jax.devices("cpu")[0]
        with jax.default_device(cpu):
            logits = jnp.asarray(xf) @ jnp.asarray(gate_w).T
            probs = jax.nn.softmax(logits.astype(jnp.float32), axis=-1)
            _, sel = jax.lax.top_k(probs, TOPK)
        return np.asarray(probs), np.asarray(sel)
    except Exception:
        logits = xf @ gate_w.T
        m = logits.max(axis=-1, keepdims=True)
        e = np.exp(logits - m, dtype=np.float32)
        probs = e / e.sum(axis=-1, keepdims=True)
        sel = np.argsort(-probs, axis=-1, kind="stable")[:, :TOPK]
        return probs, sel


def _split8(a, s):
    """-> (hi, lo) fp8 arrays with a*s ~= hi + lo."""
    sa = (a * s).astype(np.float32)
    hi = sa.astype(F8NP)
    lo = (sa - hi.astype(np.float32)).astype(F8NP)
    return hi, lo


def _slab_major(w):
    """[H, O] -> [O, H] slab-major: row ot*128+p holds slab ot's (k, o) run
    contiguously, so each w1 slab DMA moves 2048B-contiguous rows."""
    Hd, O = w.shape
    return np.ascontiguousarray(
        w.reshape(Hd // 128, 128, O // 128, 128).transpose(2, 1, 0, 3)
        .reshape(O, Hd))


def kernel(x, gate_w, w1, w2, shared_w1, shared_w2):
    x = np.asarray(x, np.float32)
    gate_w = np.asarray(gate_w, np.float32)
    w1 = np.asarray(w1, np.float32)
    w2 = np.asarray(w2, np.float32)
    shared_w1 = np.asarray(shared_w1, np.float32)
    shared_w2 = np.asarray(shared_w2, np.float32)

    B, S, Hd = x.shape
    xf = np.ascontiguousarray(x.reshape(-1, Hd))  # [T, H]

    probs, sel = _route(xf, gate_w)
    onehot = np.zeros((T, E), bool)
    onehot[np.arange(T)[:, None], sel] = True
    # sort each expert's tokens by routing weight (descending) so the first
    # token tiles hold the high-weight tokens that get full 3-product
    # precision; low-weight tokens use the cheap hi-only products
    idx_e = []
    for e in range(E):
        ix = np.nonzero(onehot[:, e])[0]
        idx_e.append(ix[np.argsort(-probs[ix, e], kind="stable")])
    counts = np.array([len(ix) for ix in idx_e])

    cap = CAP0
    while counts.max() > cap:
        cap += 64
    if cap not in _compiled:
        _compiled[cap] = _build(cap, order=(0, 2, 3, 1), defer_parts=(),
                                hi_tiles=1)
    nc = _compiled[cap]

    # quantize x once: [T, H] hi/lo, packed [H, T, 2]
    xq_hi, xq_lo = _split8(xf, SX)
    xt2 = np.empty((H, T, 2), F8NP)
    xt2[:, :, 0] = xq_hi.T
    xt2[:, :, 1] = xq_lo.T

    def pack_w1(hi, lo, n_go):
        """[H, 2*n_go*128] hi/lo -> [n_go, 4, 128, H] o-tile groups
        (gwh, gwl, uwh, uwl), each slab-major."""
        hs = _slab_major(hi).reshape(2 * n_go, 128, H)
        ls = _slab_major(lo).reshape(2 * n_go, 128, H)
        out = np.empty((n_go, 4, 128, H), F8NP)
        out[:, 0] = hs[:n_go]
        out[:, 1] = ls[:n_go]
        out[:, 2] = hs[n_go:]
        out[:, 3] = ls[n_go:]
        return out.reshape(4 * n_go * 128, H)

    cwcols = (cap + 127) // 128
    in_maps = []
    for c in range(NCORES):
        m = {"xt": xt2}
        for j in range(2):
            e = 2 * c + j
            ix = idx_e[e]
            xs2 = np.zeros((H, cap, 2), F8NP)
            xs2[:, :len(ix), 0] = xq_hi[ix].T
            xs2[:, :len(ix), 1] = xq_lo[ix].T
            m[f"xs{j}"] = xs2
            hi, lo = _split8(w1[e].T, SW1)  # [H, I2]
            m[f"w1a{j}"] = pack_w1(hi, lo, 11)
            hi, lo = _split8(w2[e].T, SW2)  # [I, H]
            w2a = np.zeros((3072, H), F8NP)
            w2a[:I] = hi
            w2a[I:1536] = hi[-128:]  # dup of k-tile 10 for the odd-k leftover
            w2a[1536:1536 + I] = lo
            m[f"w2a{j}"] = w2a
            cw = np.zeros(cwcols * 128, np.float32)
            cw[: len(ix)] = probs[ix, e] * S2_EVICT
            m[f"cw{j}"] = cw
        sl = slice(SSL * c, SSL * (c + 1))
        sg = np.zeros((H, SSLP), np.float32)
        su = np.zeros((H, SSLP), np.float32)
        sg[:, :SSL] = shared_w1[sl].T
        su[:, :SSL] = shared_w1[ISH + SSL * c: ISH + SSL * (c + 1)].T
        hi_g, lo_g = _split8(sg, SW1)
        hi_u, lo_u = _split8(su, SW1)
        m["sw1a"] = pack_w1(np.concatenate([hi_g, hi_u], axis=1),
                            np.concatenate([lo_g, lo_u], axis=1), 3)
        s2w = np.zeros((512, H), np.float32)
        s2w[:SSL] = shared_w2[:, sl].T
        hi, lo = _split8(s2w, SW2)
        hi[SSLP:] = hi[SSLP - 128: SSLP]  # dup k-tile 2
        lo[SSLP:] = 0
        m["sw2a"] = np.concatenate([hi, lo], axis=0)
        in_maps.append(m)

    try:
        res = run_bass_kernel_spmd(nc, in_maps, list(range(NCORES)))
    except ModuleNotFoundError:
        os.environ["BASS_NEVER_TRACE"] = "1"
        res = run_bass_kernel_spmd(nc, in_maps, list(range(NCORES)))
    global last_result
    last_result = res

    out = np.zeros((T, H), np.float32)
    for c in range(NCORES):
        out += res.results[c]["ys"].astype(np.float32)
        for j in range(2):
            e = 2 * c + j
            ix = idx_e[e]
            out[ix] += res.results[c][f"y{j}"][: len(ix)].astype(np.float32)

    return out.reshape(B, S, Hd)


# revision 73
# speedup vs baseline: 1.9341x; 1.0243x over previous
"""DeepseekMoE layer on 8 TRN2 NeuronCores — expert-parallel Bass/Tile kernel.

Strategy (self-contained, shapes hardcoded for this problem):
  H=2048, T=2048 tokens, E=16 experts, top-6, I=1408, shared IS=2816.

  Sharding (done on host inside kernel(), per the full-input contract):
    - Router (softmax + top-6) computed on host in fp32 (jax-on-CPU when
      available so near-tie selections match the jax reference bitwise)
      -> per-expert token lists (the "all-to-all dispatch" decision).
    - Core c owns experts 2c, 2c+1 (capacity-padded to CAP tokens each);
      shared expert sharded over its intermediate dim (352 rows per core,
      padded to 384 = 3*128).
    - Each core returns per-expert outputs [CAP, H] bf16 (pre-scaled by
      routing weights) and a dense shared partial [T, H] bf16; the host
      scatter-adds in fp32.

  Arithmetic: all matmuls run as fp8(e4m3) DoubleRow pairs at 0.5 cyc/row,
  using a hi+lo residual decomposition of every operand:
      a*s ~= a_hi + a_lo   (a_hi = fp8(a*s), a_lo = fp8(a*s - a_hi))
      a*b*s_a*s_b ~= a_hi*b_hi + a_lo*b_hi + a_hi*b_lo     (lo*lo dropped)
  Three DoubleRow instructions per k-tile pair = 0.75 cyc/row/k-tile, a
  1.33x speedup over bf16/fp32r with ~bf16 effective precision (measured
  end-to-end rel err 2.6e-3 vs the 2e-2 gate).

  Scales: x*4, w1*64, w2*128. Stage-1 PSUM: gate = g*256 (Silu evicted with
  scale 2^-8); up = u*256, fused DVE (ps_u * 1/16) * silu_g = act*16, which
  is split hi/lo to fp8 for stage 2. Stage-2 PSUM = y*2048; eviction scale
  folds 2^-11 into the per-token routing weight (or a constant for the
  shared expert). Odd k-tile counts (11 expert, 3 shared) are handled by
  host-side slab rows: hi-slab gets a duplicated last k-tile, lo-slab gets
  zeros, plus a one-time memset of the act tile's pad k-tile.
"""

import os
import sys

sys.path.insert(0, "/opt/trn_rl_repo")

import numpy as np
import ml_dtypes

import concourse.bass as bass  # noqa: F401
import concourse.tile as tile
from concourse import bacc, mybir
from concourse.bass_utils import run_bass_kernel_spmd

H = 2048
T = 2048
E = 16
TOPK = 6
I2 = 2816  # 2*I
I = 1408
ISH = 2816  # shared intermediate (per gate/up half)
NCORES = 8
CAP0 = 832  # per-expert token capacity; grown in 64s if exceeded
SSL = 352  # shared-intermediate slice per core
SSLP = 384  # padded to 3*128

SX, SW1, SW2, SACT = 4.0, 64.0, 128.0, 16.0
S1_EVICT = 1.0 / (SW1 * SX)  # 2^-8
S1_UP = SACT / (SW1 * SX)  # 1/16
S2_EVICT = 1.0 / (SACT * SW2)  # 2^-11

F8NP = ml_dtypes.float8_e4m3
BF16NP = ml_dtypes.bfloat16
F32 = mybir.dt.float32
F8 = mybir.dt.float8e4
BF16 = mybir.dt.bfloat16
AF = mybir.ActivationFunctionType
ALU = mybir.AluOpType
DR = mybir.MatmulPerfMode.DoubleRow

_compiled = {}
last_result = None  # BassKernelResults of the most recent run (for profiling)


def _nchunks(n, first=None):
    """Split n into <=512 free-dim chunks; optional smaller first chunk so the
    first PSUM group starts after a fraction of the x block has landed."""
    out = [first] if first else []
    n -= first or 0
    while n > 0:
        w = min(512, n)
        out.append(w)
        n -= w
    return out


def _dma_split_k(nc, out_tile, src, nsplit, hipri=None):
    """Split a [128, K, ...] transfer into nsplit k-range DMAs so it spreads
    across DMA engines (each engine moves ~22.5 B/ns; one big transfer is
    single-engine latency-bound)."""
    K = out_tile.shape[1]
    step = (K + nsplit - 1) // nsplit
    for k0 in range(0, K, step):
        k1 = min(k0 + step, K)
        if hipri is not None:
            with hipri.high_priority():
                nc.sync.dma_start(out=out_tile[:, k0:k1], in_=src[:, k0:k1])
        else:
            nc.sync.dma_start(out=out_tile[:, k0:k1], in_=src[:, k0:k1])


def _emit_mm3(nc, ps, wh_slab, wl_slab, x_tile, xc0, w, n_kt, first, last,
              use_wlo=True):
    """Accumulate sum_k w~[k].T @ x~[k] into ps[:, :w] via 3-product fp8
    DoubleRow instructions. wh/wl slabs: [128, n_kt(+pad), 128]; x_tile:
    [128, n_kt, 2, tokens] (hi/lo interleaved), token cols [xc0, xc0+w).
    n_kt may be odd: slabs carry a dup/zero pad k-tile (see module doc).
    use_wlo=False drops the w-residual product (2-product mode)."""
    xs = x_tile[:, :, xc0:xc0 + w, :]  # [128, kt, w, 2(hi/lo)]
    npair = n_kt // 2
    per = 3 if use_wlo else 2
    n3 = npair * per + (2 if n_kt % 2 else 0)
    i = 0
    for kp in range(npair):
        k = 2 * kp
        prods = [
            (wh_slab[:, k:k + 2, :], xs[:, k:k + 2, :, 0]),
            (wh_slab[:, k:k + 2, :], xs[:, k:k + 2, :, 1]),
        ]
        if use_wlo:
            prods.append((wl_slab[:, k:k + 2, :], xs[:, k:k + 2, :, 0]))
        for lhsT, rhs in prods:
            nc.tensor.matmul(ps[:, :w], lhsT, rhs,
                             start=(first and i == 0),
                             stop=(last and i == n3 - 1), perf_mode=DR)
            i += 1
    if n_kt % 2:
        k = n_kt - 1
        # (w_hi[k], w_hi[k]-dup) x (x_hi[k], x_lo[k]): slot dim from hi/lo
        nc.tensor.matmul(ps[:, :w], wh_slab[:, k:k + 2, :],
                         xs[:, k, :, :].rearrange("p w s -> p s w"),
                         start=(first and i == 0), stop=False, perf_mode=DR)
        # (w_lo[k], zeros) x (x_hi[k], x_hi[k+1]-garbage*0)
        nc.tensor.matmul(ps[:, :w], wl_slab[:, k:k + 2, :],
                         xs[:, k:k + 2, :, 0],
                         start=False, stop=last, perf_mode=DR)


def _emit_s1(nc, pools, *, b, x_tile, act_tile, chunks, ogroup=None):
    """Stage 1: per gate-o-tile, compute ps_g/ps_u via 3-product DoubleRow,
    then evict: ACT Silu -> ag; DVE (ps_u/16)*ag -> stage; ACT cast -> act_hi;
    DVE stage-hi -> act_lo. act_tile: [128, act_kt, 2, ntok] fp8.

    ogroup: if set, loop chunk-outer within o-groups of that size so the PE
    rides the incoming x stream instead of stalling o-by-o (startup block)."""
    w1p, psp, stp = pools["w1"], pools["ps"], pools["stage"]
    tc = pools["tc"]
    n_go = b["n_go"]
    spans = []
    t0 = 0
    for w in chunks:
        spans.append((t0, w))
        t0 += w

    hi_t = b.get("hi_t")  # tokens < hi_t: full 3-product; rest: hi-only w

    def load_slabs(o, hipri):
        # w1a rows o*512..o*512+512 hold the o-tile's 4 slabs (gwh, gwl,
        # uwh, uwl), each slab-major with its (k, o) run contiguous (2048B):
        # one DMA loads everything the o-tile needs
        t = w1p.tile([128, 4, 16, 128], F8, tag="w1slab",
                     name=f"w1_{b['tag']}_{o}")
        src = b["w1a"][o * 512:(o + 1) * 512, :].rearrange(
            "(s p) c -> p s c", p=128)
        if hipri:
            with tc.high_priority():  # gate/up halves land independently
                nc.sync.dma_start(out=t[:, 0:2], in_=src[:, 0:2])
                nc.sync.dma_start(out=t[:, 2:4], in_=src[:, 2:4])
        else:
            nc.sync.dma_start(out=t[:], in_=src)
        return [t[:, 0], t[:, 1], t[:, 2], t[:, 3]]

    def emit_o_chunk(o, slabs, ci, t0, w):
        gwh, gwl, uwh, uwl = slabs
        full = hi_t is None or t0 < hi_t
        ps_g = psp.tile([128, 512], F32, tag="ps", name=f"psg_{o}_{ci}")
        ps_u = psp.tile([128, 512], F32, tag="ps", name=f"psu_{o}_{ci}")
        _emit_mm3(nc, ps_g, gwh, gwl, x_tile, t0, w, 16, True, True,
                  use_wlo=full)
        _emit_mm3(nc, ps_u, uwh, uwl, x_tile, t0, w, 16, True, True,
                  use_wlo=full)
        ag = stp.tile([128, 512], F32, tag="stage", name=f"ag_{o}_{ci}")
        st = stp.tile([128, 512], F32, tag="stage", name=f"st_{o}_{ci}")
        nc.scalar.activation(out=ag[:, :w], in_=ps_g[:, :w], func=AF.Silu,
                             scale=S1_EVICT)
        nc.vector.scalar_tensor_tensor(
            out=st[:, :w], in0=ps_u[:, :w], scalar=S1_UP, in1=ag[:, :w],
            op0=ALU.mult, op1=ALU.mult)
        hi = act_tile[:, o, 0, t0:t0 + w]
        nc.scalar.activation(out=hi, in_=st[:, :w], func=AF.Copy)
        if full:  # act_lo only consumed by the full-precision s2 tiles
            nc.vector.tensor_sub(act_tile[:, o, 1, t0:t0 + w], st[:, :w], hi)

    if ogroup is None:
        for o in range(n_go):
            slabs = load_slabs(o, o <= 2 and b.get("hipri_slab"))
            for ci, (t0, w) in enumerate(spans):
                emit_o_chunk(o, slabs, ci, t0, w)
    else:
        for g0 in range(0, n_go, ogroup):
            os_ = range(g0, min(g0 + ogroup, n_go))
            slabs = {o: load_slabs(o, g0 == 0 and b.get("hipri_slab"))
                     for o in os_}
            for ci, (t0, w) in enumerate(spans):
                for o in os_:
                    emit_o_chunk(o, slabs[o], ci, t0, w)


def _prefetch_w2(nc, pools, b):
    """Load all of a block's w2 slabs (4 hc, hi+lo packed in one DMA each);
    emitted right after the block's s1 so they don't queue behind the next
    block's x transfers."""
    w2p = pools["w2"]
    kt = b["act_kt"]
    w2a_r = b["w2a"].rearrange("(s k p) h -> p s k h", s=2, p=128)
    res = b["w2_res"] if b.get("w2_res") is not None else b.setdefault(
        "w2_cache", [None] * 4)
    for hc in range(4):
        if res[hc] is not None:
            continue
        tag, bufs = ("w2res", 8) if b.get("w2_res") is not None else \
            ("w2slab", 6)
        sh = w2p.tile([128, kt, 512], F8, tag=tag, bufs=bufs,
                      name=f"w2h_{b['tag']}_{hc}")
        sl = w2p.tile([128, kt, 512], F8, tag=tag, bufs=bufs,
                      name=f"w2l_{b['tag']}_{hc}")
        nc.sync.dma_start(out=sh[:],
                          in_=w2a_r[:, 0, :, hc * 512:(hc + 1) * 512])
        nc.sync.dma_start(out=sl[:],
                          in_=w2a_r[:, 1, :, hc * 512:(hc + 1) * 512])
        res[hc] = (sh, sl)
    return res


def _emit_s2(nc, pools, *, b, act_tile, part):
    """Stage 2: out[t, hc] = sum_k act~[k].T @ w2~[k], 3-product DoubleRow
    with act (hi/lo) stationary and w2 slabs moving. Evict with per-token
    (expert) or constant (shared) scale to bf16, DMA out."""
    psp, outp = pools["ps2"], pools["out"]
    n_kt = b["act_kt"] - 1 if b["odd_kt"] else b["act_kt"]
    ntok = b["ntok"]
    hi_t = b.get("hi_t")
    slabs = _prefetch_w2(nc, pools, b)
    ntt = (ntok + 127) // 128
    nt1 = (ntt + 1) // 2
    tt_list = {1: range(nt1), 2: range(nt1, ntt)}[part]
    for tt in tt_list:
        r0 = tt * 128
        w = min(128, ntok - r0)
        full = hi_t is None or r0 < hi_t
        ysb = outp.tile([128, 2048], BF16, tag="ysb",
                        name=f"ysb_{b['tag']}_{tt}")
        for hc in range(4):
            w2h_slab, w2l_slab = slabs[hc]
            ps = psp.tile([128, 512], F32, tag="ps2", name=f"ps2_{hc}_{tt}")
            npair = n_kt // 2
            per = 3 if full else 1
            n3 = npair * per + (n_kt % 2) * (2 if full else 1)
            i = 0
            for kp in range(npair):
                k = 2 * kp
                prods = [
                    (act_tile[:, k:k + 2, 0, r0:r0 + w], w2h_slab[:, k:k + 2, :]),
                ]
                if full:
                    prods.append((act_tile[:, k:k + 2, 1, r0:r0 + w],
                                  w2h_slab[:, k:k + 2, :]))
                    prods.append((act_tile[:, k:k + 2, 0, r0:r0 + w],
                                  w2l_slab[:, k:k + 2, :]))
                for lhsT, rhs in prods:
                    nc.tensor.matmul(ps[:w, :], lhsT, rhs, start=(i == 0),
                                     stop=(i == n3 - 1), perf_mode=DR)
                    i += 1
            if n_kt % 2:
                k = n_kt - 1
                if full:
                    # (act_hi[k], act_lo[k]) x (w2h[k], w2h[k]-dup)
                    nc.tensor.matmul(ps[:w, :], act_tile[:, k, :, r0:r0 + w],
                                     w2h_slab[:, k:k + 2, :], start=False,
                                     stop=False, perf_mode=DR)
                    # (act_hi[k], pad-0) x (w2l[k], 0)
                    nc.tensor.matmul(ps[:w, :],
                                     act_tile[:, k:k + 2, 0, r0:r0 + w],
                                     w2l_slab[:, k:k + 2, :],
                                     start=False, stop=True, perf_mode=DR)
                else:
                    # (act_hi[k], pad-0) x (w2h[k], w2h[k]-dup): pad slot is 0
                    nc.tensor.matmul(ps[:w, :],
                                     act_tile[:, k:k + 2, 0, r0:r0 + w],
                                     w2h_slab[:, k:k + 2, :],
                                     start=False, stop=True, perf_mode=DR)
            # alternate evict engine so PSUM drain never paces the PE
            dst = ysb[:w, hc * 512:(hc + 1) * 512]
            if b["cw"] is not None:
                if hc % 2 == 0:
                    nc.scalar.activation(out=dst, in_=ps[:w, :], func=AF.Copy,
                                         scale=b["cw"][:w, tt:tt + 1])
                else:
                    nc.vector.tensor_scalar_mul(dst, ps[:w, :],
                                                b["cw"][:w, tt:tt + 1])
            else:
                if hc % 2 == 0:
                    nc.scalar.activation(out=dst, in_=ps[:w, :], func=AF.Copy,
                                         scale=S2_EVICT)
                else:
                    nc.vector.tensor_scalar_mul(dst, ps[:w, :], S2_EVICT)
        nc.sync.dma_start(
            out=b["out"][b["row0"] + r0: b["row0"] + r0 + w, :],
            in_=ysb[:w, :])


def _build(cap, order=(0, 1, 2, 3), first_chunk=256, defer_parts=(2,),
           ogroup0=None, w1bufs=3, hi_tiles=1):
    nc = bacc.Bacc("TRN2", target_bir_lowering=False, debug=False)

    cwcols = (cap + 127) // 128
    aps = {}
    for j in range(2):
        aps[f"xs{j}"] = nc.dram_tensor(f"xs{j}", [H, cap, 2], F8,
                                       kind="ExternalInput").ap()
        aps[f"w1a{j}"] = nc.dram_tensor(f"w1a{j}", [2 * I2, H], F8,
                                        kind="ExternalInput").ap()
        aps[f"w2a{j}"] = nc.dram_tensor(f"w2a{j}", [3072, H], F8,
                                        kind="ExternalInput").ap()
        aps[f"cw{j}"] = nc.dram_tensor(f"cw{j}", [cwcols * 128], F32,
                                       kind="ExternalInput").ap()
        aps[f"y{j}"] = nc.dram_tensor(f"y{j}", [cap, H], BF16,
                                      kind="ExternalOutput").ap()
    aps["xt"] = nc.dram_tensor("xt", [H, T, 2], F8, kind="ExternalInput").ap()
    aps["sw1a"] = nc.dram_tensor("sw1a", [4 * SSLP, H], F8,
                                 kind="ExternalInput").ap()
    aps["sw2a"] = nc.dram_tensor("sw2a", [1024, H], F8,
                                 kind="ExternalInput").ap()
    aps["ys"] = nc.dram_tensor("ys", [T, H], BF16, kind="ExternalOutput").ap()

    import contextlib
    with tile.TileContext(nc) as tc, contextlib.ExitStack() as ctx:
        pools = {
            "x": ctx.enter_context(tc.tile_pool(name="x", bufs=2)),
            "w1": ctx.enter_context(tc.tile_pool(name="w1", bufs=w1bufs)),
            "w2": ctx.enter_context(tc.tile_pool(name="w2", bufs=4)),
            "act": ctx.enter_context(tc.tile_pool(name="act", bufs=2)),
            "stage": ctx.enter_context(tc.tile_pool(name="stage", bufs=4)),
            "out": ctx.enter_context(tc.tile_pool(name="out", bufs=3)),
            # separate s1/s2 PSUM pools: the cross-block s2 deferral must
            # never be starved of PSUM slots by the next block's stalled s1
            "ps": ctx.enter_context(tc.tile_pool(name="ps", bufs=4,
                                                 space="PSUM")),
            "ps2": ctx.enter_context(tc.tile_pool(name="ps2", bufs=4,
                                                  space="PSUM")),
            "misc": ctx.enter_context(tc.tile_pool(name="misc", bufs=2)),
        }
        pools["tc"] = tc
        cw_tiles = {}

        def get_cw(j):
            if j not in cw_tiles:
                cw_r = aps[f"cw{j}"].rearrange("(n p) -> p n", p=128)
                cw_tiles[j] = pools["misc"].tile([128, cwcols], F32,
                                                 tag=f"cw{j}", name=f"cw{j}_t")
                nc.sync.dma_start(out=cw_tiles[j][:], in_=cw_r[:])
            return cw_tiles[j]



        shared_res = [None, None, None, None]
        all_blocks = []
        for j in range(2):
            all_blocks.append(dict(
                tag=f"e{j}", n_go=11, act_kt=12, odd_kt=True, ntok=cap,
                x_ap=aps[f"xs{j}"], x_off=0,
                w1a=aps[f"w1a{j}"], w2a=aps[f"w2a{j}"],
                out=aps[f"y{j}"], row0=0, cw_j=j, w2_res=None,
                hipri_slab=True,
                hi_t=None if hi_tiles is None else 128 * hi_tiles,
            ))
        for half in range(2):
            all_blocks.append(dict(
                tag=f"sh{half}", n_go=3, act_kt=4, odd_kt=True, ntok=1024,
                x_ap=aps["xt"], x_off=half * 1024,
                w1a=aps["sw1a"], w2a=aps["sw2a"],
                out=aps["ys"], row0=half * 1024, cw_j=None,
                w2_res=shared_res, hipri_slab=True,
            ))

        blocks = [all_blocks[i] for i in order]

        def s1_chunks(n):
            b = blocks[n]
            hi_t = b.get("hi_t")
            if hi_t:  # chunk boundary must align with the precision boundary
                rem = b["ntok"] - hi_t
                if rem > 512:  # near-equal halves keep the x-stream wait low
                    first = min(512, (rem // 2 + 63) // 64 * 64)
                    return [hi_t] + _nchunks(rem, first=first)
                return [hi_t] + _nchunks(rem)
            return _nchunks(b["ntok"], first=first_chunk if n == 0 else None)

        def load_x(n, first_hipri=False):
            # only block 0's first chunk is urgent; later blocks' x loads are
            # prefetches that must NOT outrank the current block's stream
            b = blocks[n]
            x_r = b["x_ap"].rearrange("(k p) t s -> p k t s", p=128)
            xt_tile = pools["x"].tile([128, 16, b["ntok"], 2], F8, tag="xsel",
                                      name=f"x_{b['tag']}")
            t0 = 0
            for ci, w in enumerate(s1_chunks(n)):
                src = x_r[:, :, b["x_off"] + t0: b["x_off"] + t0 + w, :]
                dst = xt_tile[:, :, t0:t0 + w, :]
                if ci == 0 and first_hipri:
                    with tc.high_priority():
                        nc.sync.dma_start(out=dst, in_=src)
                else:
                    nc.sync.dma_start(out=dst, in_=src)
                t0 += w
            return xt_tile

        def emit_s2_part(b, act_tile, part):
            if "cw" not in b:
                b["cw"] = None if b["cw_j"] is None else get_cw(b["cw_j"])
            _emit_s2(nc, pools, b=b, act_tile=act_tile, part=part)

        # Emit s1(n), then block n+1's x-load, then the previous block's
        # deferred s2 half, then s2(n) part 1: the next x-load overlaps s2(n)
        # compute instead of queueing behind its weight slabs.
        x_tiles = [load_x(0, first_hipri=True)]
        get_cw(0)  # small; emitted after block 0's x so they don't delay it
        get_cw(1)
        deferred = None
        for n, b in enumerate(blocks):
            act_tile = pools["act"].tile([128, b["act_kt"], 2, b["ntok"]], F8,
                                         tag="act", name=f"act_{b['tag']}")
            # zero the pad k-tile (hi slot is read by the odd-k leftover
            # instruction; lo slot never read)
            nc.gpsimd.memset(act_tile[:, b["act_kt"] - 1, 0, :], 0.0)
            _emit_s1(nc, pools, b=b, x_tile=x_tiles[n], act_tile=act_tile,
                     chunks=s1_chunks(n), ogroup=ogroup0 if n == 0 else None)
            _prefetch_w2(nc, pools, b)
            if n + 1 < len(blocks):
                x_tiles.append(load_x(n + 1))
            if deferred is not None:
                for p in defer_parts:
                    emit_s2_part(*deferred, part=p)
                deferred = None
            for p in (1, 2):
                if p not in defer_parts:
                    emit_s2_part(b, act_tile, part=p)
            deferred = (b, act_tile)
        if deferred is not None:
            for p in defer_parts:
                emit_s2_part(*deferred, part=p)

    nc.compile()
    return nc


def _route(xf, gate_w):
    """Host router: fp32 softmax + top-6, matching jax bitwise when possible."""
    try:
        import jax
        import jax.numpy as jnp

        cpu = jax.devices("cpu")[0]
        with jax.default_device(cpu):
            logits = jnp.asarray(xf) @ jnp.asarray(gate_w).T
            probs = jax.nn.softmax(logits.astype(jnp.float32), axis=-1)
            _, sel = jax.lax.top_k(probs, TOPK)
        return np.asarray(probs), np.asarray(sel)
    except Exception:
        logits = xf @ gate_w.T
        m = logits.max(axis=-1, keepdims=True)
        e = np.exp(logits - m, dtype=np.float32)
        probs = e / e.sum(axis=-1, keepdims=True)
        sel = np.argsort(-probs, axis=-1, kind="stable")[:, :TOPK]
        return probs, sel


def _split8(a, s):
    """-> (hi, lo) fp8 arrays with a*s ~= hi + lo."""
    sa = (a * s).astype(np.float32)
    hi = sa.astype(F8NP)
    lo = (sa - hi.astype(np.float32)).astype(F8NP)
    return hi, lo


def _slab_major(w):
    """[H, O] -> [O, H] slab-major: row ot*128+p holds slab ot's (k, o) run
    contiguously, so each w1 slab DMA moves 2048B-contiguous rows."""
    Hd, O = w.shape
    return np.ascontiguousarray(
        w.reshape(Hd // 128, 128, O // 128, 128).transpose(2, 1, 0, 3)
        .reshape(O, Hd))


def kernel(x, gate_w, w1, w2, shared_w1, shared_w2):
    x = np.asarray(x, np.float32)
    gate_w = np.asarray(gate_w, np.float32)
    w1 = np.asarray(w1, np.float32)
    w2 = np.asarray(w2, np.float32)
    shared_w1 = np.asarray(shared_w1, np.float32)
    shared_w2 = np.asarray(shared_w2, np.float32)

    B, S, Hd = x.shape
    xf = np.ascontiguousarray(x.reshape(-1, Hd))  # [T, H]

    probs, sel = _route(xf, gate_w)
    onehot = np.zeros((T, E), bool)
    onehot[np.arange(T)[:, None], sel] = True
    # sort each expert's tokens by routing weight (descending) so the first
    # token tiles hold the high-weight tokens that get full 3-product
    # precision; low-weight tokens use the cheap hi-only products
    idx_e = []
    for e in range(E):
        ix = np.nonzero(onehot[:, e])[0]
        idx_e.append(ix[np.argsort(-probs[ix, e], kind="stable")])
    counts = np.array([len(ix) for ix in idx_e])

    cap = CAP0
    while counts.max() > cap:
        cap += 64
    if cap not in _compiled:
        _compiled[cap] = _build(cap, order=(0, 1, 2, 3), defer_parts=(),
                                hi_tiles=1)
    nc = _compiled[cap]

    # quantize x once: [T, H] hi/lo, packed [H, T, 2]
    xq_hi, xq_lo = _split8(xf, SX)
    xt2 = np.empty((H, T, 2), F8NP)
    xt2[:, :, 0] = xq_hi.T
    xt2[:, :, 1] = xq_lo.T

    def pack_w1(hi, lo, n_go):
        """[H, 2*n_go*128] hi/lo -> [n_go, 4, 128, H] o-tile groups
        (gwh, gwl, uwh, uwl), each slab-major."""
        hs = _slab_major(hi).reshape(2 * n_go, 128, H)
        ls = _slab_major(lo).reshape(2 * n_go, 128, H)
        out = np.empty((n_go, 4, 128, H), F8NP)
        out[:, 0] = hs[:n_go]
        out[:, 1] = ls[:n_go]
        out[:, 2] = hs[n_go:]
        out[:, 3] = ls[n_go:]
        return out.reshape(4 * n_go * 128, H)

    cwcols = (cap + 127) // 128
    in_maps = []
    for c in range(NCORES):
        m = {"xt": xt2}
        for j in range(2):
            e = 2 * c + j
            ix = idx_e[e]
            xs2 = np.zeros((H, cap, 2), F8NP)
            xs2[:, :len(ix), 0] = xq_hi[ix].T
            xs2[:, :len(ix), 1] = xq_lo[ix].T
            m[f"xs{j}"] = xs2
            hi, lo = _split8(w1[e].T, SW1)  # [H, I2]
            m[f"w1a{j}"] = pack_w1(hi, lo, 11)
            hi, lo = _split8(w2[e].T, SW2)  # [I, H]
            w2a = np.zeros((3072, H), F8NP)
            w2a[:I] = hi
            w2a[I:1536] = hi[-128:]  # dup of k-tile 10 for the odd-k leftover
            w2a[1536:1536 + I] = lo
            m[f"w2a{j}"] = w2a
            cw = np.zeros(cwcols * 128, np.float32)
            cw[: len(ix)] = probs[ix, e] * S2_EVICT
            m[f"cw{j}"] = cw
        sl = slice(SSL * c, SSL * (c + 1))
        sg = np.zeros((H, SSLP), np.float32)
        su = np.zeros((H, SSLP), np.float32)
        sg[:, :SSL] = shared_w1[sl].T
        su[:, :SSL] = shared_w1[ISH + SSL * c: ISH + SSL * (c + 1)].T
        hi_g, lo_g = _split8(sg, SW1)
        hi_u, lo_u = _split8(su, SW1)
        m["sw1a"] = pack_w1(np.concatenate([hi_g, hi_u], axis=1),
                            np.concatenate([lo_g, lo_u], axis=1), 3)
        s2w = np.zeros((512, H), np.float32)
        s2w[:SSL] = shared_w2[:, sl].T
        hi, lo = _split8(s2w, SW2)
        hi[SSLP:] = hi[SSLP - 128: SSLP]  # dup k-tile 2
        lo[SSLP:] = 0
        m["sw2a"] = np.concatenate([hi, lo], axis=0)
        in_maps.append(m)

    try:
        res = run_bass_kernel_spmd(nc, in_maps, list(range(NCORES)))
    except ModuleNotFoundError:
        os.environ["BASS_NEVER_TRACE"] = "1"
        res = run_bass_kernel_spmd(nc, in_maps, list(range(NCORES)))
    global last_result
    last_result = res

    out = np.zeros((T, H), np.float32)
    for c in range(NCORES):
        out += res.results[c]["ys"].astype(np.float32)
        for j in range(2):
            e = 2 * c + j
            ix = idx_e[e]
            out[ix] += res.results[c][f"y{j}"][: len(ix)].astype(np.float32)

    return out.reshape(B, S, Hd)
